# revision 66
# baseline (speedup 1.0000x reference)
"""AttentionBlock (GroupNorm -> qkv -> 8-head attention -> proj -> residual)
as a distributed Bass/Tile kernel on 8 TRN2 NeuronCores.

Sharding: pure data-parallel over batch B=8 -> one batch element per core,
zero collectives. Each core computes its whole attention block.

Per-core algorithm (C=512, L=1024, NH=8, ch=64, G=32 groups):
  - x shipped bf16, host-pre-transposed to [128, 4*L] (fat contiguous
    per-partition descriptors), issued 2 dma_starts per chunk BEFORE the
    weight DMAs (each dma_start maps to one ~26GB/s HW-DGE queue).
  - GroupNorm stats: chunks 0-2 via DVE bn_stats, chunk 3 via ScalarE
    Copy/Square activations with accum_out (parallel engines; square sits
    in the same ACT table as ln/exp); tiny PE matmuls reduce channel stats
    to group stats (16 channels/group) and broadcast back. gamma/beta
    folded into the qkv weights host-side, attention scale (ch^-1/4 on q
    and k) folded into the q weights as 1/sqrt(ch); GN apply split DVE /
    ScalarE (activation with per-partition scale+bias).
  - qkv / v / AV / proj matmuls in fp8 e4m3 with MatmulPerfMode.DoubleRow
    (2 k-tiles per pass, ~2.2x measured PE throughput); scores in bf16
    (64-deep contraction cannot DoubleRow). q,k drained to bf16; xn, vT,
    softmax probabilities and hid kept in fp8.
  - scores computed TRANSPOSED: sT[s, t] = k^T q (lhsT=k, rhs=q); the two
    heads of a pair sit at partitions 0:64 / 64:128. v produced directly
    transposed by swapping matmul operands -- no on-chip transposes.
  - softmax computed SHIFTED: p' = exp(s - 2.5) (cancels in p'/D'), which
    keeps the fp8 exp in range. Exp split ScalarE table Exp (f8 out) /
    DVE Schraudolph writing fp8 BITS via uint8 bitcast (float->uint
    saturates negatives to 0 = fp8 +0 on deep underflow). GPSIMD cannot
    read PSUM on this HW, so only these two engines can run the exp.
  - AV lhsT is [v | ones*64], so rows 64:128 of the AV PSUM tile hold the
    softmax denominator D replicated across 64 partitions; division is
    ScalarE exp(-ln(D)) (Ln/Exp share one ACT table with the softmax Exp,
    no table switch) + one DVE multiply.
  - software-pipelined schedule: scores(0)/(1) emitted inside the qkv
    phase so the exp stream overlaps qkv; scores(3) halves interleaved
    with AV(2) tiles so the PE always has independent DoubleRow work
    queued when scores stall on the exp/PSUM rotation; b_proj folded into
    the proj drain (scalar_tensor_tensor).
  - PE warm-up drip: dummy matmuls during the DMA/GN head keep the Tensor
    engine p-state ramped (2.4GHz warm vs 1.2GHz after idle).
  - proj drained per 512-col half, output written bf16 (host upcasts).
"""

import sys
import types

import numpy as np
import ml_dtypes

BF16 = ml_dtypes.bfloat16
F8 = ml_dtypes.float8_e4m3

C = 512
L = 1024
NH = 8
CH = 64
G = 32
EPS = 1e-5
N_CORES = 8


# ---------------------------------------------------------------------------
# Environment compat (inlined so kernel.py is self-contained)
# ---------------------------------------------------------------------------
def _install_compat():
    # 1) NTFF profiling hook shim (image's antenv stub lacks axon_hooks).
    try:
        from antenv.axon_hooks import get_axon_ntff_profile_hook  # noqa: F401
    except ImportError:
        try:
            import antenv
            from trn_agent_boot.trn_boot import _ntff_profile_via_ctypes

            m = types.ModuleType("antenv.axon_hooks")
            m._hook = None
            m.set_axon_ntff_profile_hook = lambda h: setattr(m, "_hook", h)
            m.get_axon_ntff_profile_hook = lambda: m._hook
            sys.modules["antenv.axon_hooks"] = m
            antenv.axon_hooks = m
            m.set_axon_ntff_profile_hook(
                _ntff_profile_via_ctypes("/opt/axon/libaxon_pjrt.so")
            )
        except Exception:
            pass

    # 2) gpsimd.sem_clear over a wide semaphore range exceeds this walrus
    #    build's ISA payload limit ("ISA wrong length"); chunk the clears.
    import concourse.bass as bass

    if not getattr(bass.Bass.clear_and_free_semaphores, "_chunk_patch", False):
        _orig_clear = bass.Bass.clear_and_free_semaphores

        def _chunked_clear(self, sems, _orig=_orig_clear):
            sems = list(sems)
            for i in range(0, len(sems), 4):
                _orig(self, sems[i : i + 4])

        _chunked_clear._chunk_patch = True
        bass.Bass.clear_and_free_semaphores = _chunked_clear


def _split_waits(nc):
    """This walrus build accepts at most ONE semaphore wait per instruction;
    Tile emits up to 2 (and the closing drain more). Split the extras into
    standalone EVENT_SEM instructions inserted just before, on the same
    engine, which is semantically identical (same-engine program order)."""
    from concourse import mybir

    nid = 0
    for blk in nc.m.functions[0].blocks:
        new_list = []
        for inst in blk.instructions:
            si = inst.sync_info
            if si and si.on_wait and len(si.on_wait) > 1:
                waits = list(si.on_wait)
                si.on_wait = waits[-1:]
                for w in waits[:-1]:
                    nid += 1
                    ev = mybir.InstEventSemaphore(
                        name=f"WSPLIT-{nid}", ins=[], outs=[]
                    )
                    ev.engine = inst.engine
                    ev.sync_info = mybir.SyncInfo(on_wait=[w], on_update=[])
                    nc.register_instruction(ev, overwrite=True)
                    new_list.append(ev)
            new_list.append(inst)
        blk.instructions[:] = new_list


# ---------------------------------------------------------------------------
# Bass graph
# ---------------------------------------------------------------------------
# Schraudolph exp constants (bf16 bit-space)
A16 = 184.6649652337873
B16 = 16250.37
# Schraudolph exp constants (fp8 e4m3 bit-space); the softmax is computed
# shifted, p' = exp(s - CSHIFT), which cancels in p'/sum(p') -- keeps the
# fp8 exp under the e4m3 finite max (240) for |s| <= ~8 and keeps typical
# probabilities in the normal range.
A8 = 11.541560327111707
B8 = 55.65
CSHIFT = 2.5

# exp engine maps: per (st, hh) -> "dve", default ScalarE. (GPSIMD cannot
# read PSUM on this HW, so only ScalarE/DVE can run the softmax exp.)
# j=0..2 run inside the qkv phase (ScalarE-heavy); j=3 runs in the AV tail
# where DVE must carry the division too.
EXP_EARLY = {  # j = 0, 1, 2
    (1, 0): "dve", (3, 1): "dve", (5, 0): "dve", (6, 1): "dve", (7, 0): "dve",
}
EXP_LATE = {  # j = 3
    (1, 0): "dve", (3, 1): "dve", (5, 0): "dve", (6, 1): "dve",
    (0, 1): "dve", (2, 0): "dve", (4, 1): "dve", (7, 0): "dve",
}

DEFAULT_CFG = dict(
    reorder=True,        # kc-outer/nh-inner in qkv/proj
    no_exp=False,        # probe: skip softmax exp (timing only)
    no_div=False,        # probe: skip division chain
    no_gn=False,         # probe: skip GroupNorm stats chain (memset scales)
    psum3=True,          # scores tag A bufs=3; av split by nh to [128, 512]
    xdbuf=True,          # x_sb in a bufs=2 pool (loop mode overlap)
    swpipe=True,         # software-pipeline attention: scores(j+1) before AV(j)
    ptbufs=5,            # pt pool buffers
    early=True,          # scores(0)/(1) inside the qkv phase
    div="lnexp",         # "lnexp" (ScalarE exp(-ln(D))) | "recip" (DVE, slow)
                         # ("approx" custom-DVE recip hits this walrus
                         # build's "ISA wrong length" limit)
    warm=(12, 2, 2),     # PE warmup matmul counts (pre-GN, mid-GN, post-GN);
                         # w1=20 measured worse (warmups sit ahead of the
                         # GN group matmul in PE order and delay it); w2/w3
                         # sit on the PE critical path so keep them tiny
    gn_pool=0,           # GN-apply chunks on Pool (Pool is ~20x slower for
                         # tensor ops on this HW -- memsets only)
    bp_fuse=True,        # fold x += b_proj into the proj drain (one
                         # scalar_tensor_tensor op: (ps + bp) + x)
    drain_split=True,    # proj drain + out DMA per 512-col half
    obf16=True,          # output DRAM tensor in bf16 (host upcasts)
    early3=False,        # scores(0..2) inside the qkv phase (with the
                         # DVE-heavy exp maps, 2 early scores measured best)
    v_ilv=False,         # interleave scores(2) halves with the v phase
                         # (measured WORSE: s2's DVE exps block the v
                         # drains queued behind them -- priority inversion)
    fp8=True,            # qkv/v/AV/proj matmuls in fp8 e4m3 DoubleRow
                         # (2.2x PE throughput); scores stay bf16
)


def build_nc(loop_n=None, cfg=None):
    import concourse.bass as bass
    import concourse.tile as tile
    from concourse import mybir

    cfg = {**DEFAULT_CFG, **(cfg or {})}
    i16 = mybir.dt.int16
    u8 = mybir.dt.uint8
    f32 = mybir.dt.float32
    bf = mybir.dt.bfloat16
    f8 = mybir.dt.float8e4
    PM = mybir.MatmulPerfMode
    AF = mybir.ActivationFunctionType
    OP = mybir.AluOpType
    FP8 = cfg["fp8"]
    adt = f8 if FP8 else bf  # activation dtype for xn/v/pts/hid

    nc = bass.Bass(trn_type="TRN2")
    xd = nc.declare_dram_parameter("x", [128, 4 * L], bf, isOutput=False)
    wqkd = nc.declare_dram_parameter("wqk", [128, 8 * C], adt, isOutput=False)
    wvd = nc.declare_dram_parameter("wv", [128, 4 * C], adt, isOutput=False)
    wpd = nc.declare_dram_parameter("wp", [128, 4 * C], adt, isOutput=False)
    bqkd = nc.declare_dram_parameter("bqk", [128, 8], f32, isOutput=False)
    bvd = nc.declare_dram_parameter("bvb", [128, C], bf, isOutput=False)
    bpd = nc.declare_dram_parameter("bp", [128, 4], f32, isOutput=False)
    indd = nc.declare_dram_parameter("ind", [128, 8], f32, isOutput=False)
    indTd = nc.declare_dram_parameter("indT", [8, 128], f32, isOutput=False)
    odt = bf if cfg["obf16"] else f32
    outd = nc.declare_dram_parameter("out", [C, L], odt, isOutput=True)

    with tile.TileContext(nc) as tc:
        with (
            tc.tile_pool(name="cst", bufs=1) as cst,
            tc.tile_pool(name="act", bufs=1) as actp,
            tc.tile_pool(name="xbp", bufs=2) as xbp,
            tc.tile_pool(name="ptp", bufs=cfg["ptbufs"]) as ptp,
            tc.tile_pool(name="dnp", bufs=4) as dnp,
            tc.tile_pool(name="otp", bufs=8) as otp,
            tc.tile_pool(name="psp", bufs=1, space="PSUM") as psp,
        ):
            # warmup constant (memset on Pool: cheap and dep-free)
            wrm = cst.tile([128, 512], bf)
            nc.gpsimd.memset(wrm, 0.125)
            # per-partition -CSHIFT bias tile for the shifted fp8 exp
            csh = cst.tile([128, 1], f32)
            nc.gpsimd.memset(csh, -CSHIFT)
            # dependency-free dummy activation: forces the ACT_TABLE_LOAD
            # (1.3us) to run at engine boot instead of lazily before the
            # first real activation (which waits on the x DMA)
            twarm = cst.tile([128, 1], f32)
            nc.scalar.activation(out=twarm, in_=csh, func=AF.Exp)

            def emit_warm(n):
                for _ in range(n):
                    wp_ = psp.tile([128, 512], f32, tag="B", bufs=2)
                    nc.tensor.matmul(
                        wp_, lhsT=wrm[:, 0:128], rhs=wrm, start=True, stop=True
                    )

            def alloc_x():
                xpool = xbp if cfg["xdbuf"] else actp
                x_sb = xpool.tile([128, 4, L], bf)
                xr = xd[:, :].rearrange("p (a o) -> p a o", o=L)
                for t in range(4):
                    # 2 dma_starts per chunk: each dma_start is one ~26GB/s
                    # HW-DGE queue, so this halves the x landing time
                    nc.sync.dma_start(
                        out=x_sb[0:64, t, :], in_=xr[0:64, t, :]
                    )
                    nc.sync.dma_start(
                        out=x_sb[64:128, t, :], in_=xr[64:128, t, :]
                    )
                return x_sb

            def emit_weight_loads():
                wqk_sb = cst.tile([128, 4, 2 * C], adt)
                nc.sync.dma_start(
                    out=wqk_sb,
                    in_=wqkd[:, :].rearrange("p (a o) -> p a o", o=2 * C),
                )
                wv_sb = cst.tile([128, 4, C], adt)
                nc.sync.dma_start(
                    out=wv_sb, in_=wvd[:, :].rearrange("p (a o) -> p a o", o=C)
                )
                wp_sb = cst.tile([128, 4, C], adt)
                nc.sync.dma_start(
                    out=wp_sb, in_=wpd[:, :].rearrange("p (a o) -> p a o", o=C)
                )
                bqk_sb = cst.tile([128, 8], f32)
                nc.sync.dma_start(out=bqk_sb, in_=bqkd[:, :])
                bvb_sb = cst.tile([128, C], bf)
                nc.sync.dma_start(out=bvb_sb, in_=bvd[:, :])
                bp_sb = cst.tile([128, 4], f32)
                nc.sync.dma_start(out=bp_sb, in_=bpd[:, :])
                ind_sb = cst.tile([128, 8], f32)
                nc.sync.dma_start(out=ind_sb, in_=indd[:, :])
                indT_sb = cst.tile([8, 128], f32)
                nc.sync.dma_start(out=indT_sb, in_=indTd[:, :])
                eps8 = cst.tile([8, 1], f32)
                nc.vector.memset(eps8, EPS)
                return (wqk_sb, wv_sb, wp_sb, bqk_sb, bvb_sb, bp_sb, ind_sb,
                        indT_sb, eps8)

            def _emit_body(ws, x_pre=None):
                (wqk_sb, wv_sb, wp_sb, bqk_sb, bvb_sb, bp_sb, ind_sb,
                 indT_sb, eps8) = ws
                x_sb = x_pre if x_pre is not None else alloc_x()

                xn_sb = actp.tile([128, 4, L], adt)
                q_sb = actp.tile([128, 4, L], bf)
                k_sb = actp.tile([128, 4, L], bf)
                VW = 2 * CH
                vT_sb = actp.tile([128, 8, NH, VW], adt)
                hid_sb = actp.tile([128, 4, L], adt)
                sc_sb = actp.tile([128, 8], f32)
                if cfg["no_div"]:
                    for t in range(4):
                        nc.vector.memset(hid_sb[:, t, :], 0.001)

                if cfg["warm"][0]:
                    emit_warm(cfg["warm"][0])

                # ---- GroupNorm statistics. stats4 cols (2t, 2t+1) hold
                # (mean, E[x^2]) per channel: chunks 0-2 via DVE bn_stats,
                # chunk 3 via ScalarE Copy/Square with accum_out (the engines
                # run in parallel; square shares the ln/exp ACT table)
                st6 = actp.tile([128, 3, 2, 6], f32)
                mv = actp.tile([128, 3, 2], f32)
                stats4 = actp.tile([128, 8], f32)
                sqs = actp.tile([128, L], bf)
                if cfg["no_gn"]:
                    nc.vector.memset(sc_sb, 1.0)
                # ScalarE takes chunk 0 (the x chunks land ~1.2us apart in
                # DMA issue order, so chunk 0 is ready first and the slower
                # 2-pass ScalarE pipeline gets the head start); DVE bn_stats
                # take chunks 1-3 as they land
                if not cfg["no_gn"]:
                    nc.scalar.activation(
                        out=sqs, in_=x_sb[:, 0, :], func=AF.Copy,
                        scale=1.0 / L, accum_out=stats4[:, 0:1],
                    )
                    nc.scalar.activation(
                        out=sqs, in_=x_sb[:, 0, :], func=AF.Square,
                        scale=1.0 / 32.0, accum_out=stats4[:, 1:2],
                    )
                for t in [] if cfg["no_gn"] else range(1, 4):
                    for s in range(2):
                        nc.vector.bn_stats(
                            out=st6[:, t - 1, s, :],
                            in_=x_sb[:, t, 512 * s : 512 * (s + 1)],
                        )
                    nc.vector.bn_aggr(
                        out=mv[:, t - 1, :], in_=st6[:, t - 1, :, :]
                    )
                if not cfg["no_gn"]:
                    s4 = stats4.rearrange("p (t s) -> p t s", s=2)
                    nc.vector.tensor_copy(out=s4[:, 1:4, 0], in_=mv[:, :, 0])
                    nc.vector.tensor_mul(
                        out=s4[:, 1:4, 1], in0=mv[:, :, 0], in1=mv[:, :, 0]
                    )
                    nc.vector.tensor_add(
                        out=s4[:, 1:4, 1], in0=s4[:, 1:4, 1], in1=mv[:, :, 1]
                    )
                    gmm = psp.tile([8, 8], f32, tag="B", bufs=2)
                    nc.tensor.matmul(
                        gmm, lhsT=ind_sb, rhs=stats4, start=True, stop=True
                    )
                    if cfg["warm"][1]:
                        emit_warm(cfg["warm"][1])
                    gm = actp.tile([8, 8], f32)
                    nc.vector.tensor_scalar_mul(out=gm, in0=gmm, scalar1=1.0 / 16.0)
                    gmr = gm.rearrange("g (t s) -> g t s", s=2)
                    msq = actp.tile([8, 4], f32)
                    nc.vector.tensor_mul(out=msq, in0=gmr[:, :, 0], in1=gmr[:, :, 0])
                    gv = actp.tile([8, 4], f32)
                    nc.vector.tensor_tensor(
                        out=gv, in0=gmr[:, :, 1], in1=msq, op=OP.subtract
                    )
                    # rsqrt(v + eps) = exp(-0.5 * ln(v + eps)): Log and Exp
                    # share one ACT table set, so the softmax Exp later needs
                    # no table switch.
                    sd = actp.tile([8, 4], f32)
                    nc.scalar.activation(out=sd, in_=gv, func=AF.Ln, bias=eps8, scale=1.0)
                    inv8 = actp.tile([8, 4], f32)
                    nc.scalar.activation(out=inv8, in_=sd, func=AF.Exp, scale=-0.5)
                    sh8 = actp.tile([8, 4], f32)
                    nc.vector.tensor_mul(out=sh8, in0=gmr[:, :, 0], in1=inv8)
                    nc.vector.tensor_scalar_mul(out=sh8, in0=sh8, scalar1=-1.0)
                    gs = actp.tile([8, 8], f32)
                    gsr = gs.rearrange("g (t s) -> g t s", s=2)
                    nc.vector.tensor_copy(out=gsr[:, :, 0], in_=inv8)
                    nc.vector.tensor_copy(out=gsr[:, :, 1], in_=sh8)
                    nb = psp.tile([128, 8], f32, tag="B", bufs=2)
                    nc.tensor.matmul(nb, lhsT=indT_sb, rhs=gs, start=True, stop=True)
                    nc.vector.tensor_copy(out=sc_sb, in_=nb)
                if cfg["warm"][2]:
                    emit_warm(cfg["warm"][2])
                for t in range(4):
                    # split applies across DVE and ScalarE (per-partition
                    # scale/bias activation) so they finish in half the time
                    if t >= 2:
                        nc.scalar.activation(
                            out=xn_sb[:, t, :],
                            in_=x_sb[:, t, :],
                            func=AF.Identity,
                            scale=sc_sb[:, 2 * t : 2 * t + 1],
                            bias=sc_sb[:, 2 * t + 1 : 2 * t + 2],
                        )
                    else:
                        nc.vector.tensor_scalar(
                            out=xn_sb[:, t, :],
                            in0=x_sb[:, t, :],
                            scalar1=sc_sb[:, 2 * t : 2 * t + 1],
                            scalar2=sc_sb[:, 2 * t + 1 : 2 * t + 2],
                            op0=OP.mult,
                            op1=OP.add,
                        )

                if not cfg["bp_fuse"]:
                    # residual base: x + b_proj (in place)
                    for m in range(4):
                        nc.vector.tensor_scalar_add(
                            out=x_sb[:, m, :], in0=x_sb[:, m, :],
                            scalar1=bp_sb[:, m : m + 1],
                        )

                # ---- qkv: q,k in natural [c, l] layout
                def emit_qkv_m(m):
                    dst = q_sb if m < 4 else k_sb
                    # nh-split 1-bank tag-B tiles so the scores pss (tag A)
                    # rotation never blocks qkv
                    for nh in range(2):
                        ps = psp.tile([128, 512], f32, tag="B", bufs=2)
                        if FP8:
                            for u in range(2):
                                nc.tensor.matmul(
                                    ps,
                                    lhsT=wqk_sb[:, 2 * u : 2 * u + 2,
                                                128 * m : 128 * (m + 1)],
                                    rhs=xn_sb[:, 2 * u : 2 * u + 2,
                                              512 * nh : 512 * (nh + 1)],
                                    start=(u == 0),
                                    stop=(u == 1),
                                    perf_mode=PM.DoubleRow,
                                )
                        else:
                            for kc in range(4):
                                nc.tensor.matmul(
                                    ps,
                                    lhsT=wqk_sb[:, kc, 128 * m : 128 * (m + 1)],
                                    rhs=xn_sb[:, kc, 512 * nh : 512 * (nh + 1)],
                                    start=(kc == 0),
                                    stop=(kc == 3),
                                )
                        nc.vector.tensor_scalar_add(
                            out=dst[:, m % 4, 512 * nh : 512 * (nh + 1)],
                            in0=ps, scalar1=bqk_sb[:, m : m + 1],
                        )

                def emit_v_phase(lts=range(8), ones=True):
                    # v, produced directly transposed: vT[l, c] + ones cols
                    # (ones blocks on Pool: off the critical path)
                    if ones:
                        for lt in range(8):
                            nc.gpsimd.memset(vT_sb[:, lt, :, CH:VW], 1.0)
                    for lt in lts:
                        ps = psp.tile([128, C], f32, tag="B", bufs=2)
                        if FP8:
                            for u in range(2):
                                nc.tensor.matmul(
                                    ps,
                                    lhsT=xn_sb[:, 2 * u : 2 * u + 2,
                                               128 * lt : 128 * (lt + 1)],
                                    rhs=wv_sb[:, 2 * u : 2 * u + 2, :],
                                    start=(u == 0),
                                    stop=(u == 1),
                                    perf_mode=PM.DoubleRow,
                                )
                        else:
                            for kc in range(4):
                                nc.tensor.matmul(
                                    ps,
                                    lhsT=xn_sb[:, kc, 128 * lt : 128 * (lt + 1)],
                                    rhs=wv_sb[:, kc, :],
                                    start=(kc == 0),
                                    stop=(kc == 3),
                                )
                        nc.vector.tensor_tensor(
                            out=vT_sb[:, lt, :, 0:CH],
                            in0=ps.rearrange("p (h c) -> p h c", h=NH),
                            in1=bvb_sb.rearrange("p (h c) -> p h c", h=NH),
                            op=OP.add,
                        )

                # ---- attention, head pairs (2j at partitions 0:64,
                #      2j+1 at 64:128)
                def emit_scores(j, pts=None, sts=None, emap=None):
                    m = j
                    n_early = 3 if cfg["early3"] else 2
                    if emap is None:
                        emap = (
                            EXP_EARLY if (cfg["early"] and j < n_early)
                            else EXP_LATE
                        )
                    if pts is None:
                        pts = [
                            ptp.tile([128, 8, L], adt, tag="pt", name=f"pt{j}_0"),
                            ptp.tile([128, 8, L], adt, tag="pt", name=f"pt{j}_1"),
                        ]
                    if cfg["no_exp"] and sts is None:
                        for st in range(8):
                            nc.vector.memset(pts[0][:, st, :], 0.001)
                            nc.vector.memset(pts[1][:, st, :], 0.001)
                    for st in (range(8) if sts is None else sts):
                        abufs = 3 if cfg["psum3"] else 2
                        pss = [
                            psp.tile([128, L], f32, tag="A", bufs=abufs,
                                     name=f"qkt{j}_{st}_0"),
                            psp.tile([128, L], f32, tag="A", bufs=abufs,
                                     name=f"qkt{j}_{st}_1"),
                        ]
                        # sequential per-head emission measured faster on HW
                        # than interleaving the pair's matmuls
                        for hh in range(2):
                            for nh in range(2):
                                po = 64 * hh
                                nc.tensor.matmul(
                                    pss[hh][:, 512 * nh : 512 * (nh + 1)],
                                    lhsT=k_sb[po : po + 64, m, 128 * st : 128 * (st + 1)],
                                    rhs=q_sb[po : po + 64, m, 512 * nh : 512 * (nh + 1)],
                                    start=True,
                                    stop=True,
                                )
                        for hh in [] if cfg["no_exp"] else range(2):
                            eng = emap.get((st, hh))
                            if eng == "dve":
                                # u8 bitcast: float->uint saturates negatives
                                # to 0, so deep-underflow exps land on f8 +0
                                # instead of wrapping to garbage
                                iv = pts[hh][:, st, :].bitcast(
                                    u8 if FP8 else i16
                                )
                                sa, sb = (A8, B8 - A8 * CSHIFT) if FP8 else (
                                    A16, B16
                                )
                                with nc.allow_low_precision(
                                    reason="Schraudolph exp ripple ok for softmax"
                                ):
                                    nc.vector.tensor_scalar(
                                        out=iv, in0=pss[hh],
                                        scalar1=sa, scalar2=sb,
                                        op0=OP.mult, op1=OP.add,
                                    )
                            else:
                                nc.scalar.activation(
                                    out=pts[hh][:, st, :], in_=pss[hh],
                                    func=AF.Exp,
                                    bias=csh if FP8 else 0.0,
                                )
                    return pts

                def emit_div(av, j, hh, nh, m):
                    if cfg["no_div"]:
                        return
                    dst = hid_sb[
                        64 * hh : 64 * hh + CH, m, 512 * nh : 512 * (nh + 1)
                    ]
                    if cfg["div"] == "approx":
                        dinv = dnp.tile([CH, 512], f32, tag="dsb")
                        nc.vector.reciprocal_approx_fast(
                            out=dinv, in_=av[CH:128, :]
                        )
                    elif cfg["div"] == "lnexp":
                        lnb = dnp.tile([CH, 512], f32, tag="lnb")
                        nc.scalar.activation(
                            out=lnb, in_=av[CH:128, :], func=AF.Ln
                        )
                        dinv = dnp.tile([CH, 512], bf, tag="dsb")
                        nc.scalar.activation(
                            out=dinv, in_=lnb, func=AF.Exp, scale=-1.0
                        )
                    else:
                        dinv = dnp.tile([CH, 512], bf, tag="dsb")
                        with nc.allow_low_precision(
                            reason="softmax 1/D in bf16 is within tolerance"
                        ):
                            nc.vector.reciprocal(out=dinv, in_=av[CH:128, :])
                    nc.vector.tensor_mul(out=dst, in0=av[0:CH, :], in1=dinv)

                def emit_av(j, pts, hhs=(0, 1)):
                    m = j
                    # finer AV granularity: one [128, 512] PSUM tile per
                    # (hh, nh); the division of one tile overlaps the AV
                    # matmuls of the next
                    for hh in hhs:
                        for nh in range(2):
                            av = psp.tile(
                                [128, 512], f32, tag="B", bufs=2,
                                name=f"av{j}_{hh}_{nh}",
                            )
                            if FP8:
                                for u in range(4):
                                    nc.tensor.matmul(
                                        av,
                                        lhsT=vT_sb[:, 2 * u : 2 * u + 2,
                                                   2 * j + hh, :],
                                        rhs=pts[hh][:, 2 * u : 2 * u + 2,
                                                    512 * nh : 512 * (nh + 1)],
                                        start=(u == 0),
                                        stop=(u == 3),
                                        perf_mode=PM.DoubleRow,
                                    )
                            else:
                                for st in range(8):
                                    nc.tensor.matmul(
                                        av,
                                        lhsT=vT_sb[:, st, 2 * j + hh, :],
                                        rhs=pts[hh][:, st,
                                                    512 * nh : 512 * (nh + 1)],
                                        start=(st == 0),
                                        stop=(st == 7),
                                    )
                            emit_div(av, j, hh, nh, m)

                if cfg["early"] and cfg["early3"]:
                    # scores(0..2) emitted inside the qkv phase: the exp
                    # stream overlaps the qkv matmuls; only scores(3)'s exps
                    # remain in the AV tail
                    emit_qkv_m(0)
                    emit_qkv_m(4)
                    pts0 = emit_scores(0)
                    emit_qkv_m(1)
                    emit_qkv_m(5)
                    pts1 = emit_scores(1)
                    emit_qkv_m(2)
                    emit_qkv_m(6)
                    pts2 = emit_scores(2)
                    emit_qkv_m(3)
                    emit_qkv_m(7)
                    emit_v_phase()
                    emit_av(0, pts0)
                    pts3 = emit_scores(3)
                    emit_av(1, pts1)
                    emit_av(2, pts2)
                    emit_av(3, pts3)
                elif cfg["early"]:
                    emit_qkv_m(0)
                    emit_qkv_m(4)
                    pts0 = emit_scores(0)
                    emit_qkv_m(1)
                    emit_qkv_m(5)
                    pts1 = emit_scores(1)
                    for m_ in (2, 6, 3, 7):
                        emit_qkv_m(m_)
                    if cfg["v_ilv"]:
                        # interleave scores(2) halves with the v phase so
                        # its exp stream starts during the v matmuls
                        pts2 = emit_scores(2, sts=range(0, 4), emap=EXP_EARLY)
                        emit_v_phase(lts=range(0, 4))
                        emit_scores(2, pts=pts2, sts=range(4, 8),
                                    emap=EXP_EARLY)
                        emit_v_phase(lts=range(4, 8), ones=False)
                        emit_av(0, pts0)
                        emit_av(1, pts1)
                    else:
                        emit_v_phase()
                        emit_av(0, pts0)
                        pts2 = emit_scores(2)
                        emit_av(1, pts1)
                    # interleave scores(3) halves with av(2) tiles so the PE
                    # has independent DR work queued when the scores matmuls
                    # stall on the tag-A/exp rotation
                    pts3 = emit_scores(3, sts=range(0, 4))
                    emit_av(2, pts2, hhs=(0,))
                    emit_scores(3, pts=pts3, sts=range(4, 8))
                    emit_av(2, pts2, hhs=(1,))
                    emit_av(3, pts3)
                elif cfg["swpipe"]:
                    for m_ in (0, 4, 1, 5, 2, 6, 3, 7):
                        emit_qkv_m(m_)
                    emit_v_phase()
                    ptss = {0: emit_scores(0), 1: emit_scores(1)}
                    emit_av(0, ptss.pop(0))
                    ptss[2] = emit_scores(2)
                    emit_av(1, ptss.pop(1))
                    ptss[3] = emit_scores(3)
                    emit_av(2, ptss.pop(2))
                    emit_av(3, ptss.pop(3))
                else:
                    for m_ in (0, 4, 1, 5, 2, 6, 3, 7):
                        emit_qkv_m(m_)
                    emit_v_phase()
                    for j in range(4):
                        emit_av(j, emit_scores(j))

                # ---- proj + residual (residual base x+b_p precomputed)
                for m in range(4):
                    ps = psp.tile(
                        [128, L], f32, tag="A", bufs=3 if cfg["psum3"] else 2,
                        name=f"proj{m}",
                    )
                    if FP8:
                        for u, nh in [(u, nh) for u in range(2) for nh in range(2)]:
                            nc.tensor.matmul(
                                ps[:, 512 * nh : 512 * (nh + 1)],
                                lhsT=wp_sb[:, 2 * u : 2 * u + 2,
                                           128 * m : 128 * (m + 1)],
                                rhs=hid_sb[:, 2 * u : 2 * u + 2,
                                           512 * nh : 512 * (nh + 1)],
                                start=(u == 0),
                                stop=(u == 1),
                                perf_mode=PM.DoubleRow,
                            )
                    else:
                        loops = (
                            [(kc, nh) for kc in range(4) for nh in range(2)]
                            if cfg["reorder"]
                            else [(kc, nh) for nh in range(2) for kc in range(4)]
                        )
                        for kc, nh in loops:
                            nc.tensor.matmul(
                                ps[:, 512 * nh : 512 * (nh + 1)],
                                lhsT=wp_sb[:, kc, 128 * m : 128 * (m + 1)],
                                rhs=hid_sb[:, kc, 512 * nh : 512 * (nh + 1)],
                                start=(kc == 0),
                                stop=(kc == 3),
                            )
                    def drain_col(c0, c1):
                        ob = otp.tile([128, c1 - c0], odt, tag="ob")
                        if cfg["bp_fuse"]:
                            nc.vector.scalar_tensor_tensor(
                                out=ob,
                                in0=ps[:, c0:c1],
                                scalar=bp_sb[:, m : m + 1],
                                in1=x_sb[:, m, c0:c1],
                                op0=OP.add,
                                op1=OP.add,
                            )
                        else:
                            nc.vector.tensor_add(
                                out=ob, in0=ps[:, c0:c1], in1=x_sb[:, m, c0:c1]
                            )
                        nc.sync.dma_start(
                            out=outd[128 * m : 128 * (m + 1), c0:c1], in_=ob
                        )

                    if cfg["drain_split"]:
                        drain_col(0, 512)
                        drain_col(512, 1024)
                    else:
                        drain_col(0, 1024)

            if loop_n:
                ws = emit_weight_loads()
                with tc.For_i(0, loop_n, 1):
                    _emit_body(ws)
            else:
                x_sb = alloc_x()
                ws = emit_weight_loads()
                _emit_body(ws, x_pre=x_sb)

    _split_waits(nc)
    return nc


_NC = None


def _get_nc():
    global _NC
    if _NC is None:
        _install_compat()
        _NC = build_nc()
    return _NC


def _pack_rows(W, o):
    # [C, o] -> [128, 4*o]: row c = a*128 + p  ->  partition p, chunk a
    return np.ascontiguousarray(
        W.reshape(4, 128, o).transpose(1, 0, 2).reshape(128, 4 * o)
    )


def _host_prep(x, gamma, beta, w_qkv, b_qkv, w_proj, b_proj):
    x = np.asarray(x, np.float32)
    gamma = np.asarray(gamma, np.float32)
    beta = np.asarray(beta, np.float32)
    w_qkv = np.asarray(w_qkv, np.float32)
    b_qkv = np.asarray(b_qkv, np.float32)
    w_proj = np.asarray(w_proj, np.float32)
    b_proj = np.asarray(b_proj, np.float32)

    s2 = 1.0 / np.sqrt(CH)  # attention scale applied to q AND k => s^2 on q
    Wg = w_qkv * gamma[None, :]
    bb = w_qkv @ beta + b_qkv
    Wg = Wg.copy()
    Wg[0:C] *= s2
    bb = bb.copy()
    bb[0:C] *= s2

    WDT = F8 if DEFAULT_CFG["fp8"] else BF16
    shared = {
        "wqk": _pack_rows(np.ascontiguousarray(Wg[0 : 2 * C].T), 2 * C).astype(WDT),
        "wv": _pack_rows(np.ascontiguousarray(Wg[2 * C : 3 * C].T), C).astype(WDT),
        "wp": _pack_rows(np.ascontiguousarray(w_proj.T), C).astype(WDT),
        "bqk": np.ascontiguousarray(bb[0 : 2 * C].reshape(8, 128).T).astype(
            np.float32
        ),
        "bvb": np.broadcast_to(bb[2 * C : 3 * C].reshape(1, C), (128, C)).astype(
            BF16
        ),
        "bp": np.ascontiguousarray(b_proj.reshape(4, 128).T).astype(np.float32),
        "ind": (np.arange(128)[:, None] // 16 == np.arange(8)[None, :]).astype(
            np.float32
        ),
        "indT": (np.arange(128)[None, :] // 16 == np.arange(8)[:, None]).astype(
            np.float32
        ),
    }
    in_maps = []
    for b in range(N_CORES):
        m = dict(shared)
        m["x"] = _pack_rows(x[b].reshape(C, L), L).astype(BF16)
        in_maps.append(m)
    return in_maps


def run_spmd(in_maps, trace=False):
    from concourse.bass_utils import run_bass_kernel_spmd

    nc = _get_nc()
    return run_bass_kernel_spmd(
        nc, in_maps, core_ids=list(range(N_CORES)), trace=trace
    )


def kernel(x, gamma, beta, w_qkv, b_qkv, w_proj, b_proj):
    _install_compat()
    in_maps = _host_prep(x, gamma, beta, w_qkv, b_qkv, w_proj, b_proj)
    res = run_spmd(in_maps, trace=False)
    out = np.stack(
        [res.results[c]["out"].reshape(C, 32, 32) for c in range(N_CORES)]
    ).astype(np.float32)
    return out


# revision 70
# speedup vs baseline: 1.2223x; 1.2223x over previous
"""AttentionBlock (GroupNorm -> qkv -> 8-head attention -> proj -> residual)
as a distributed Bass/Tile kernel on 8 TRN2 NeuronCores.

Sharding: pure data-parallel over batch B=8 -> one batch element per core,
zero collectives. Each core computes its whole attention block.

Per-core algorithm (C=512, L=1024, NH=8, ch=64, G=32 groups):
  - x shipped bf16, host-pre-transposed to [128, 4*L] (fat contiguous
    per-partition descriptors), issued 2 dma_starts per chunk BEFORE the
    weight DMAs (each dma_start maps to one ~26GB/s HW-DGE queue).
  - GroupNorm stats: chunks 0-2 via DVE bn_stats, chunk 3 via ScalarE
    Copy/Square activations with accum_out (parallel engines; square sits
    in the same ACT table as ln/exp); tiny PE matmuls reduce channel stats
    to group stats (16 channels/group) and broadcast back. gamma/beta
    folded into the qkv weights host-side, attention scale (ch^-1/4 on q
    and k) folded into the q weights as 1/sqrt(ch); GN apply split DVE /
    ScalarE (activation with per-partition scale+bias).
  - qkv / v / AV / proj matmuls in fp8 e4m3 with MatmulPerfMode.DoubleRow
    (2 k-tiles per pass, ~2.2x measured PE throughput); scores in bf16
    (64-deep contraction cannot DoubleRow). q,k drained to bf16; xn, vT,
    softmax probabilities and hid kept in fp8.
  - scores computed TRANSPOSED: sT[s, t] = k^T q (lhsT=k, rhs=q); the two
    heads of a pair sit at partitions 0:64 / 64:128. v produced directly
    transposed by swapping matmul operands -- no on-chip transposes.
  - softmax computed SHIFTED: p' = exp(s - 2.5) (cancels in p'/D'), which
    keeps the fp8 exp in range. Exp split ScalarE table Exp (f8 out) /
    DVE Schraudolph writing fp8 BITS via uint8 bitcast (float->uint
    saturates negatives to 0 = fp8 +0 on deep underflow). GPSIMD cannot
    read PSUM on this HW, so only these two engines can run the exp.
  - AV lhsT is [v | ones*64], so rows 64:128 of the AV PSUM tile hold the
    softmax denominator D replicated across 64 partitions; division is
    ScalarE exp(-ln(D)) (Ln/Exp share one ACT table with the softmax Exp,
    no table switch) + one DVE multiply.
  - software-pipelined schedule: scores(0)/(1) emitted inside the qkv
    phase so the exp stream overlaps qkv; scores(3) halves interleaved
    with AV(2) tiles so the PE always has independent DoubleRow work
    queued when scores stall on the exp/PSUM rotation; b_proj folded into
    the proj drain (scalar_tensor_tensor).
  - PE warm-up drip: dummy matmuls during the DMA/GN head keep the Tensor
    engine p-state ramped (2.4GHz warm vs 1.2GHz after idle).
  - proj drained per 512-col half, output written bf16 (host upcasts).
"""

import sys
import types

import numpy as np
import ml_dtypes

BF16 = ml_dtypes.bfloat16
F8 = ml_dtypes.float8_e4m3

C = 512
L = 1024
NH = 8
CH = 64
G = 32
EPS = 1e-5
N_CORES = 8


# ---------------------------------------------------------------------------
# Environment compat (inlined so kernel.py is self-contained)
# ---------------------------------------------------------------------------
def _install_compat():
    # 1) NTFF profiling hook shim (image's antenv stub lacks axon_hooks).
    try:
        from antenv.axon_hooks import get_axon_ntff_profile_hook  # noqa: F401
    except ImportError:
        try:
            import antenv
            from trn_agent_boot.trn_boot import _ntff_profile_via_ctypes

            m = types.ModuleType("antenv.axon_hooks")
            m._hook = None
            m.set_axon_ntff_profile_hook = lambda h: setattr(m, "_hook", h)
            m.get_axon_ntff_profile_hook = lambda: m._hook
            sys.modules["antenv.axon_hooks"] = m
            antenv.axon_hooks = m
            m.set_axon_ntff_profile_hook(
                _ntff_profile_via_ctypes("/opt/axon/libaxon_pjrt.so")
            )
        except Exception:
            pass

    # 2) gpsimd.sem_clear over a wide semaphore range exceeds this walrus
    #    build's ISA payload limit ("ISA wrong length"); chunk the clears.
    import concourse.bass as bass

    if not getattr(bass.Bass.clear_and_free_semaphores, "_chunk_patch", False):
        _orig_clear = bass.Bass.clear_and_free_semaphores

        def _chunked_clear(self, sems, _orig=_orig_clear):
            sems = list(sems)
            for i in range(0, len(sems), 4):
                _orig(self, sems[i : i + 4])

        _chunked_clear._chunk_patch = True
        bass.Bass.clear_and_free_semaphores = _chunked_clear


def _split_waits(nc):
    """This walrus build accepts at most ONE semaphore wait per instruction;
    Tile emits up to 2 (and the closing drain more). Split the extras into
    standalone EVENT_SEM instructions inserted just before, on the same
    engine, which is semantically identical (same-engine program order)."""
    from concourse import mybir

    nid = 0
    for blk in nc.m.functions[0].blocks:
        new_list = []
        for inst in blk.instructions:
            si = inst.sync_info
            if si and si.on_wait and len(si.on_wait) > 1:
                waits = list(si.on_wait)
                si.on_wait = waits[-1:]
                for w in waits[:-1]:
                    nid += 1
                    ev = mybir.InstEventSemaphore(
                        name=f"WSPLIT-{nid}", ins=[], outs=[]
                    )
                    ev.engine = inst.engine
                    ev.sync_info = mybir.SyncInfo(on_wait=[w], on_update=[])
                    nc.register_instruction(ev, overwrite=True)
                    new_list.append(ev)
            new_list.append(inst)
        blk.instructions[:] = new_list


# ---------------------------------------------------------------------------
# Bass graph
# ---------------------------------------------------------------------------
# Schraudolph exp constants (bf16 bit-space)
A16 = 184.6649652337873
B16 = 16250.37
# Schraudolph exp constants (fp8 e4m3 bit-space); the softmax is computed
# shifted, p' = exp(s - CSHIFT), which cancels in p'/sum(p') -- keeps the
# fp8 exp under the e4m3 finite max (240) for |s| <= ~8 and keeps typical
# probabilities in the normal range.
A8 = 11.541560327111707
B8 = 55.65
CSHIFT = 2.5

# exp engine maps: per (st, hh) -> "dve", default ScalarE. (GPSIMD cannot
# read PSUM on this HW, so only ScalarE/DVE can run the softmax exp.)
# j=0..2 run inside the qkv phase (ScalarE-heavy); j=3 runs in the AV tail
# where DVE must carry the division too.
EXP_EARLY = {  # j = 0, 1, 2
    (1, 0): "dve", (3, 1): "dve", (5, 0): "dve", (6, 1): "dve", (7, 0): "dve",
}
EXP_LATE = {  # j = 3
    (1, 0): "dve", (3, 1): "dve", (5, 0): "dve", (6, 1): "dve",
    (0, 1): "dve", (2, 0): "dve", (4, 1): "dve", (7, 0): "dve",
}

DEFAULT_CFG = dict(
    reorder=True,        # kc-outer/nh-inner in qkv/proj
    no_exp=False,        # probe: skip softmax exp (timing only)
    no_div=False,        # probe: skip division chain
    no_gn=False,         # probe: skip GroupNorm stats chain (memset scales)
    psum3=True,          # scores tag A bufs=3; av split by nh to [128, 512]
    xdbuf=True,          # x_sb in a bufs=2 pool (loop mode overlap)
    swpipe=True,         # software-pipeline attention: scores(j+1) before AV(j)
    ptbufs=5,            # pt pool buffers
    early=True,          # scores(0)/(1) inside the qkv phase
    div="lnexp",         # "lnexp" (ScalarE exp(-ln(D))) | "recip" (DVE, slow)
                         # ("approx" custom-DVE recip hits this walrus
                         # build's "ISA wrong length" limit)
    warm=(12, 2, 2),     # PE warmup matmul counts (pre-GN, mid-GN, post-GN);
                         # w1=20 measured worse (warmups sit ahead of the
                         # GN group matmul in PE order and delay it); w2/w3
                         # sit on the PE critical path so keep them tiny
    gn_pool=0,           # GN-apply chunks on Pool (Pool is ~20x slower for
                         # tensor ops on this HW -- memsets only)
    bp_fuse=True,        # fold x += b_proj into the proj drain (one
                         # scalar_tensor_tensor op: (ps + bp) + x)
    drain_split=True,    # proj drain + out DMA per 512-col half
    obf16=True,          # output DRAM tensor in bf16 (host upcasts)
    early3=False,        # scores(0..2) inside the qkv phase (with the
                         # DVE-heavy exp maps, 2 early scores measured best)
    v_ilv=False,         # interleave scores(2) halves with the v phase
                         # (measured WORSE: s2's DVE exps block the v
                         # drains queued behind them -- priority inversion)
    x1dma=True,          # x as one 3D-AP dma (wide queue fan-out)
    fp8=True,            # qkv/v/AV/proj matmuls in fp8 e4m3 DoubleRow
                         # (2.2x PE throughput); scores stay bf16
)


def build_nc(loop_n=None, cfg=None):
    import concourse.bass as bass
    import concourse.tile as tile
    from concourse import mybir

    cfg = {**DEFAULT_CFG, **(cfg or {})}
    i16 = mybir.dt.int16
    u8 = mybir.dt.uint8
    f32 = mybir.dt.float32
    bf = mybir.dt.bfloat16
    f8 = mybir.dt.float8e4
    PM = mybir.MatmulPerfMode
    AF = mybir.ActivationFunctionType
    OP = mybir.AluOpType
    FP8 = cfg["fp8"]
    adt = f8 if FP8 else bf  # activation dtype for xn/v/pts/hid

    nc = bass.Bass(trn_type="TRN2")
    xd = nc.declare_dram_parameter("x", [128, 4 * L], bf, isOutput=False)
    wqkd = nc.declare_dram_parameter("wqk", [128, 8 * C], adt, isOutput=False)
    wvd = nc.declare_dram_parameter("wv", [128, 4 * C], adt, isOutput=False)
    wpd = nc.declare_dram_parameter("wp", [128, 4 * C], adt, isOutput=False)
    bqkd = nc.declare_dram_parameter("bqk", [128, 8], f32, isOutput=False)
    bvd = nc.declare_dram_parameter("bvb", [128, C], bf, isOutput=False)
    bpd = nc.declare_dram_parameter("bp", [128, 4], f32, isOutput=False)
    indd = nc.declare_dram_parameter("ind", [128, 8], f32, isOutput=False)
    indTd = nc.declare_dram_parameter("indT", [8, 128], f32, isOutput=False)
    odt = bf if cfg["obf16"] else f32
    outd = nc.declare_dram_parameter("out", [C, L], odt, isOutput=True)

    with tile.TileContext(nc) as tc:
        with (
            tc.tile_pool(name="cst", bufs=1) as cst,
            tc.tile_pool(name="act", bufs=1) as actp,
            tc.tile_pool(name="xbp", bufs=2) as xbp,
            tc.tile_pool(name="ptp", bufs=cfg["ptbufs"]) as ptp,
            tc.tile_pool(name="dnp", bufs=4) as dnp,
            tc.tile_pool(name="otp", bufs=8) as otp,
            tc.tile_pool(name="psp", bufs=1, space="PSUM") as psp,
        ):
            # warmup constant (memset on Pool: cheap and dep-free)
            wrm = cst.tile([128, 512], bf)
            nc.gpsimd.memset(wrm, 0.125)
            # per-partition -CSHIFT bias tile for the shifted fp8 exp
            csh = cst.tile([128, 1], f32)
            nc.gpsimd.memset(csh, -CSHIFT)
            # dependency-free dummy activation: forces the ACT_TABLE_LOAD
            # (1.3us) to run at engine boot instead of lazily before the
            # first real activation (which waits on the x DMA)
            twarm = cst.tile([128, 1], f32)
            nc.scalar.activation(out=twarm, in_=csh, func=AF.Exp)

            def emit_warm(n):
                for _ in range(n):
                    wp_ = psp.tile([128, 512], f32, tag="B", bufs=2)
                    nc.tensor.matmul(
                        wp_, lhsT=wrm[:, 0:128], rhs=wrm, start=True, stop=True
                    )

            def alloc_x():
                xpool = xbp if cfg["xdbuf"] else actp
                x_sb = xpool.tile([128, 4, L], bf)
                xr = xd[:, :].rearrange("p (a o) -> p a o", o=L)
                if cfg["x1dma"]:
                    # one 3D-AP dma (512 descriptors): 3D APs fan out across
                    # ~10 HW-DGE queues (observed on the weight loads), so
                    # all chunks land together and early; splitting into two
                    # 3D dmas measured WORSE (ring overhead, no extra fan)
                    nc.sync.dma_start(out=x_sb, in_=xr)
                else:
                    for t in range(4):
                        nc.sync.dma_start(
                            out=x_sb[0:64, t, :], in_=xr[0:64, t, :]
                        )
                        nc.sync.dma_start(
                            out=x_sb[64:128, t, :], in_=xr[64:128, t, :]
                        )
                return x_sb

            def emit_weight_loads():
                wqk_sb = cst.tile([128, 4, 2 * C], adt)
                nc.sync.dma_start(
                    out=wqk_sb,
                    in_=wqkd[:, :].rearrange("p (a o) -> p a o", o=2 * C),
                )
                wv_sb = cst.tile([128, 4, C], adt)
                nc.sync.dma_start(
                    out=wv_sb, in_=wvd[:, :].rearrange("p (a o) -> p a o", o=C)
                )
                wp_sb = cst.tile([128, 4, C], adt)
                nc.sync.dma_start(
                    out=wp_sb, in_=wpd[:, :].rearrange("p (a o) -> p a o", o=C)
                )
                bqk_sb = cst.tile([128, 8], f32)
                nc.sync.dma_start(out=bqk_sb, in_=bqkd[:, :])
                bvb_sb = cst.tile([128, C], bf)
                nc.sync.dma_start(out=bvb_sb, in_=bvd[:, :])
                bp_sb = cst.tile([128, 4], f32)
                nc.sync.dma_start(out=bp_sb, in_=bpd[:, :])
                ind_sb = cst.tile([128, 8], f32)
                nc.sync.dma_start(out=ind_sb, in_=indd[:, :])
                indT_sb = cst.tile([8, 128], f32)
                nc.sync.dma_start(out=indT_sb, in_=indTd[:, :])
                eps8 = cst.tile([8, 1], f32)
                nc.vector.memset(eps8, EPS)
                return (wqk_sb, wv_sb, wp_sb, bqk_sb, bvb_sb, bp_sb, ind_sb,
                        indT_sb, eps8)

            def _emit_body(ws, x_pre=None):
                (wqk_sb, wv_sb, wp_sb, bqk_sb, bvb_sb, bp_sb, ind_sb,
                 indT_sb, eps8) = ws
                x_sb = x_pre if x_pre is not None else alloc_x()

                xn_sb = actp.tile([128, 4, L], adt)
                q_sb = actp.tile([128, 4, L], bf)
                k_sb = actp.tile([128, 4, L], bf)
                VW = 2 * CH
                vT_sb = actp.tile([128, 8, NH, VW], adt)
                hid_sb = actp.tile([128, 4, L], adt)
                sc_sb = actp.tile([128, 8], f32)
                if cfg["no_div"]:
                    for t in range(4):
                        nc.vector.memset(hid_sb[:, t, :], 0.001)

                if cfg["warm"][0]:
                    emit_warm(cfg["warm"][0])

                # ---- GroupNorm statistics. stats4 cols (2t, 2t+1) hold
                # (mean, E[x^2]) per channel: chunks 0-2 via DVE bn_stats,
                # chunk 3 via ScalarE Copy/Square with accum_out (the engines
                # run in parallel; square shares the ln/exp ACT table)
                st6 = actp.tile([128, 3, 2, 6], f32)
                mv = actp.tile([128, 3, 2], f32)
                stats4 = actp.tile([128, 8], f32)
                sqs = actp.tile([128, L], bf)
                if cfg["no_gn"]:
                    nc.vector.memset(sc_sb, 1.0)
                # ScalarE takes chunk 0 (the x chunks land ~1.2us apart in
                # DMA issue order, so chunk 0 is ready first and the slower
                # 2-pass ScalarE pipeline gets the head start); DVE bn_stats
                # take chunks 1-3 as they land
                if not cfg["no_gn"]:
                    nc.scalar.activation(
                        out=sqs, in_=x_sb[:, 0, :], func=AF.Copy,
                        scale=1.0 / L, accum_out=stats4[:, 0:1],
                    )
                    nc.scalar.activation(
                        out=sqs, in_=x_sb[:, 0, :], func=AF.Square,
                        scale=1.0 / 32.0, accum_out=stats4[:, 1:2],
                    )
                for t in [] if cfg["no_gn"] else range(1, 4):
                    for s in range(2):
                        nc.vector.bn_stats(
                            out=st6[:, t - 1, s, :],
                            in_=x_sb[:, t, 512 * s : 512 * (s + 1)],
                        )
                    nc.vector.bn_aggr(
                        out=mv[:, t - 1, :], in_=st6[:, t - 1, :, :]
                    )
                if not cfg["no_gn"]:
                    s4 = stats4.rearrange("p (t s) -> p t s", s=2)
                    nc.vector.tensor_copy(out=s4[:, 1:4, 0], in_=mv[:, :, 0])
                    nc.vector.tensor_mul(
                        out=s4[:, 1:4, 1], in0=mv[:, :, 0], in1=mv[:, :, 0]
                    )
                    nc.vector.tensor_add(
                        out=s4[:, 1:4, 1], in0=s4[:, 1:4, 1], in1=mv[:, :, 1]
                    )
                    gmm = psp.tile([8, 8], f32, tag="B", bufs=2)
                    nc.tensor.matmul(
                        gmm, lhsT=ind_sb, rhs=stats4, start=True, stop=True
                    )
                    if cfg["warm"][1]:
                        emit_warm(cfg["warm"][1])
                    gm = actp.tile([8, 8], f32)
                    nc.vector.tensor_scalar_mul(out=gm, in0=gmm, scalar1=1.0 / 16.0)
                    gmr = gm.rearrange("g (t s) -> g t s", s=2)
                    msq = actp.tile([8, 4], f32)
                    nc.vector.tensor_mul(out=msq, in0=gmr[:, :, 0], in1=gmr[:, :, 0])
                    gv = actp.tile([8, 4], f32)
                    nc.vector.tensor_tensor(
                        out=gv, in0=gmr[:, :, 1], in1=msq, op=OP.subtract
                    )
                    # rsqrt(v + eps) = exp(-0.5 * ln(v + eps)): Log and Exp
                    # share one ACT table set, so the softmax Exp later needs
                    # no table switch.
                    sd = actp.tile([8, 4], f32)
                    nc.scalar.activation(out=sd, in_=gv, func=AF.Ln, bias=eps8, scale=1.0)
                    inv8 = actp.tile([8, 4], f32)
                    nc.scalar.activation(out=inv8, in_=sd, func=AF.Exp, scale=-0.5)
                    sh8 = actp.tile([8, 4], f32)
                    nc.vector.tensor_mul(out=sh8, in0=gmr[:, :, 0], in1=inv8)
                    nc.vector.tensor_scalar_mul(out=sh8, in0=sh8, scalar1=-1.0)
                    gs = actp.tile([8, 8], f32)
                    gsr = gs.rearrange("g (t s) -> g t s", s=2)
                    nc.vector.tensor_copy(out=gsr[:, :, 0], in_=inv8)
                    nc.vector.tensor_copy(out=gsr[:, :, 1], in_=sh8)
                    nb = psp.tile([128, 8], f32, tag="B", bufs=2)
                    nc.tensor.matmul(nb, lhsT=indT_sb, rhs=gs, start=True, stop=True)
                    nc.vector.tensor_copy(out=sc_sb, in_=nb)
                if cfg["warm"][2]:
                    emit_warm(cfg["warm"][2])
                for t in range(4):
                    # split applies across DVE and ScalarE (per-partition
                    # scale/bias activation) so they finish in half the time
                    if t >= 2:
                        nc.scalar.activation(
                            out=xn_sb[:, t, :],
                            in_=x_sb[:, t, :],
                            func=AF.Identity,
                            scale=sc_sb[:, 2 * t : 2 * t + 1],
                            bias=sc_sb[:, 2 * t + 1 : 2 * t + 2],
                        )
                    else:
                        nc.vector.tensor_scalar(
                            out=xn_sb[:, t, :],
                            in0=x_sb[:, t, :],
                            scalar1=sc_sb[:, 2 * t : 2 * t + 1],
                            scalar2=sc_sb[:, 2 * t + 1 : 2 * t + 2],
                            op0=OP.mult,
                            op1=OP.add,
                        )

                if not cfg["bp_fuse"]:
                    # residual base: x + b_proj (in place)
                    for m in range(4):
                        nc.vector.tensor_scalar_add(
                            out=x_sb[:, m, :], in0=x_sb[:, m, :],
                            scalar1=bp_sb[:, m : m + 1],
                        )

                # ---- qkv: q,k in natural [c, l] layout
                def emit_qkv_m(m):
                    dst = q_sb if m < 4 else k_sb
                    # nh-split 1-bank tag-B tiles so the scores pss (tag A)
                    # rotation never blocks qkv
                    for nh in range(2):
                        ps = psp.tile([128, 512], f32, tag="B", bufs=2)
                        if FP8:
                            for u in range(2):
                                nc.tensor.matmul(
                                    ps,
                                    lhsT=wqk_sb[:, 2 * u : 2 * u + 2,
                                                128 * m : 128 * (m + 1)],
                                    rhs=xn_sb[:, 2 * u : 2 * u + 2,
                                              512 * nh : 512 * (nh + 1)],
                                    start=(u == 0),
                                    stop=(u == 1),
                                    perf_mode=PM.DoubleRow,
                                )
                        else:
                            for kc in range(4):
                                nc.tensor.matmul(
                                    ps,
                                    lhsT=wqk_sb[:, kc, 128 * m : 128 * (m + 1)],
                                    rhs=xn_sb[:, kc, 512 * nh : 512 * (nh + 1)],
                                    start=(kc == 0),
                                    stop=(kc == 3),
                                )
                        nc.vector.tensor_scalar_add(
                            out=dst[:, m % 4, 512 * nh : 512 * (nh + 1)],
                            in0=ps, scalar1=bqk_sb[:, m : m + 1],
                        )

                def emit_v_phase(lts=range(8), ones=True):
                    # v, produced directly transposed: vT[l, c] + ones cols
                    # (ones blocks on Pool: off the critical path)
                    if ones:
                        for lt in range(8):
                            nc.gpsimd.memset(vT_sb[:, lt, :, CH:VW], 1.0)
                    for lt in lts:
                        ps = psp.tile([128, C], f32, tag="B", bufs=2)
                        if FP8:
                            for u in range(2):
                                nc.tensor.matmul(
                                    ps,
                                    lhsT=xn_sb[:, 2 * u : 2 * u + 2,
                                               128 * lt : 128 * (lt + 1)],
                                    rhs=wv_sb[:, 2 * u : 2 * u + 2, :],
                                    start=(u == 0),
                                    stop=(u == 1),
                                    perf_mode=PM.DoubleRow,
                                )
                        else:
                            for kc in range(4):
                                nc.tensor.matmul(
                                    ps,
                                    lhsT=xn_sb[:, kc, 128 * lt : 128 * (lt + 1)],
                                    rhs=wv_sb[:, kc, :],
                                    start=(kc == 0),
                                    stop=(kc == 3),
                                )
                        nc.vector.tensor_tensor(
                            out=vT_sb[:, lt, :, 0:CH],
                            in0=ps.rearrange("p (h c) -> p h c", h=NH),
                            in1=bvb_sb.rearrange("p (h c) -> p h c", h=NH),
                            op=OP.add,
                        )

                # ---- attention, head pairs (2j at partitions 0:64,
                #      2j+1 at 64:128)
                def emit_scores(j, pts=None, sts=None, emap=None):
                    m = j
                    n_early = 3 if cfg["early3"] else 2
                    if emap is None:
                        emap = (
                            EXP_EARLY if (cfg["early"] and j < n_early)
                            else EXP_LATE
                        )
                    if pts is None:
                        pts = [
                            ptp.tile([128, 8, L], adt, tag="pt", name=f"pt{j}_0"),
                            ptp.tile([128, 8, L], adt, tag="pt", name=f"pt{j}_1"),
                        ]
                    if cfg["no_exp"] and sts is None:
                        for st in range(8):
                            nc.vector.memset(pts[0][:, st, :], 0.001)
                            nc.vector.memset(pts[1][:, st, :], 0.001)
                    for st in (range(8) if sts is None else sts):
                        abufs = 3 if cfg["psum3"] else 2
                        pss = [
                            psp.tile([128, L], f32, tag="A", bufs=abufs,
                                     name=f"qkt{j}_{st}_0"),
                            psp.tile([128, L], f32, tag="A", bufs=abufs,
                                     name=f"qkt{j}_{st}_1"),
                        ]
                        # sequential per-head emission measured faster on HW
                        # than interleaving the pair's matmuls
                        for hh in range(2):
                            for nh in range(2):
                                po = 64 * hh
                                nc.tensor.matmul(
                                    pss[hh][:, 512 * nh : 512 * (nh + 1)],
                                    lhsT=k_sb[po : po + 64, m, 128 * st : 128 * (st + 1)],
                                    rhs=q_sb[po : po + 64, m, 512 * nh : 512 * (nh + 1)],
                                    start=True,
                                    stop=True,
                                )
                        for hh in [] if cfg["no_exp"] else range(2):
                            eng = emap.get((st, hh))
                            if eng == "dve":
                                # u8 bitcast: float->uint saturates negatives
                                # to 0, so deep-underflow exps land on f8 +0
                                # instead of wrapping to garbage
                                iv = pts[hh][:, st, :].bitcast(
                                    u8 if FP8 else i16
                                )
                                sa, sb = (A8, B8 - A8 * CSHIFT) if FP8 else (
                                    A16, B16
                                )
                                with nc.allow_low_precision(
                                    reason="Schraudolph exp ripple ok for softmax"
                                ):
                                    nc.vector.tensor_scalar(
                                        out=iv, in0=pss[hh],
                                        scalar1=sa, scalar2=sb,
                                        op0=OP.mult, op1=OP.add,
                                    )
                            else:
                                nc.scalar.activation(
                                    out=pts[hh][:, st, :], in_=pss[hh],
                                    func=AF.Exp,
                                    bias=csh if FP8 else 0.0,
                                )
                    return pts

                def emit_div(av, j, hh, nh, m):
                    if cfg["no_div"]:
                        return
                    dst = hid_sb[
                        64 * hh : 64 * hh + CH, m, 512 * nh : 512 * (nh + 1)
                    ]
                    if cfg["div"] == "approx":
                        dinv = dnp.tile([CH, 512], f32, tag="dsb")
                        nc.vector.reciprocal_approx_fast(
                            out=dinv, in_=av[CH:128, :]
                        )
                    elif cfg["div"] == "lnexp":
                        lnb = dnp.tile([CH, 512], f32, tag="lnb")
                        nc.scalar.activation(
                            out=lnb, in_=av[CH:128, :], func=AF.Ln
                        )
                        dinv = dnp.tile([CH, 512], bf, tag="dsb")
                        nc.scalar.activation(
                            out=dinv, in_=lnb, func=AF.Exp, scale=-1.0
                        )
                    else:
                        dinv = dnp.tile([CH, 512], bf, tag="dsb")
                        with nc.allow_low_precision(
                            reason="softmax 1/D in bf16 is within tolerance"
                        ):
                            nc.vector.reciprocal(out=dinv, in_=av[CH:128, :])
                    nc.vector.tensor_mul(out=dst, in0=av[0:CH, :], in1=dinv)

                def emit_av(j, pts, hhs=(0, 1)):
                    m = j
                    # finer AV granularity: one [128, 512] PSUM tile per
                    # (hh, nh); the division of one tile overlaps the AV
                    # matmuls of the next
                    for hh in hhs:
                        for nh in range(2):
                            av = psp.tile(
                                [128, 512], f32, tag="B", bufs=2,
                                name=f"av{j}_{hh}_{nh}",
                            )
                            if FP8:
                                for u in range(4):
                                    nc.tensor.matmul(
                                        av,
                                        lhsT=vT_sb[:, 2 * u : 2 * u + 2,
                                                   2 * j + hh, :],
                                        rhs=pts[hh][:, 2 * u : 2 * u + 2,
                                                    512 * nh : 512 * (nh + 1)],
                                        start=(u == 0),
                                        stop=(u == 3),
                                        perf_mode=PM.DoubleRow,
                                    )
                            else:
                                for st in range(8):
                                    nc.tensor.matmul(
                                        av,
                                        lhsT=vT_sb[:, st, 2 * j + hh, :],
                                        rhs=pts[hh][:, st,
                                                    512 * nh : 512 * (nh + 1)],
                                        start=(st == 0),
                                        stop=(st == 7),
                                    )
                            emit_div(av, j, hh, nh, m)

                if cfg["early"] and cfg["early3"]:
                    # scores(0..2) emitted inside the qkv phase: the exp
                    # stream overlaps the qkv matmuls; only scores(3)'s exps
                    # remain in the AV tail
                    emit_qkv_m(0)
                    emit_qkv_m(4)
                    pts0 = emit_scores(0)
                    emit_qkv_m(1)
                    emit_qkv_m(5)
                    pts1 = emit_scores(1)
                    emit_qkv_m(2)
                    emit_qkv_m(6)
                    pts2 = emit_scores(2)
                    emit_qkv_m(3)
                    emit_qkv_m(7)
                    emit_v_phase()
                    emit_av(0, pts0)
                    pts3 = emit_scores(3)
                    emit_av(1, pts1)
                    emit_av(2, pts2)
                    emit_av(3, pts3)
                elif cfg["early"]:
                    emit_qkv_m(0)
                    emit_qkv_m(4)
                    pts0 = emit_scores(0)
                    emit_qkv_m(1)
                    emit_qkv_m(5)
                    pts1 = emit_scores(1)
                    for m_ in (2, 6, 3, 7):
                        emit_qkv_m(m_)
                    if cfg["v_ilv"]:
                        # interleave scores(2) halves with the v phase so
                        # its exp stream starts during the v matmuls
                        pts2 = emit_scores(2, sts=range(0, 4), emap=EXP_EARLY)
                        emit_v_phase(lts=range(0, 4))
                        emit_scores(2, pts=pts2, sts=range(4, 8),
                                    emap=EXP_EARLY)
                        emit_v_phase(lts=range(4, 8), ones=False)
                        emit_av(0, pts0)
                        emit_av(1, pts1)
                    else:
                        emit_v_phase()
                        emit_av(0, pts0)
                        pts2 = emit_scores(2)
                        emit_av(1, pts1)
                    # interleave scores(3) halves with av(2) tiles so the PE
                    # has independent DR work queued when the scores matmuls
                    # stall on the tag-A/exp rotation
                    pts3 = emit_scores(3, sts=range(0, 4))
                    emit_av(2, pts2, hhs=(0,))
                    emit_scores(3, pts=pts3, sts=range(4, 8))
                    emit_av(2, pts2, hhs=(1,))
                    emit_av(3, pts3)
                elif cfg["swpipe"]:
                    for m_ in (0, 4, 1, 5, 2, 6, 3, 7):
                        emit_qkv_m(m_)
                    emit_v_phase()
                    ptss = {0: emit_scores(0), 1: emit_scores(1)}
                    emit_av(0, ptss.pop(0))
                    ptss[2] = emit_scores(2)
                    emit_av(1, ptss.pop(1))
                    ptss[3] = emit_scores(3)
                    emit_av(2, ptss.pop(2))
                    emit_av(3, ptss.pop(3))
                else:
                    for m_ in (0, 4, 1, 5, 2, 6, 3, 7):
                        emit_qkv_m(m_)
                    emit_v_phase()
                    for j in range(4):
                        emit_av(j, emit_scores(j))

                # ---- proj + residual (residual base x+b_p precomputed)
                for m in range(4):
                    ps = psp.tile(
                        [128, L], f32, tag="A", bufs=3 if cfg["psum3"] else 2,
                        name=f"proj{m}",
                    )
                    if FP8:
                        for u, nh in [(u, nh) for u in range(2) for nh in range(2)]:
                            nc.tensor.matmul(
                                ps[:, 512 * nh : 512 * (nh + 1)],
                                lhsT=wp_sb[:, 2 * u : 2 * u + 2,
                                           128 * m : 128 * (m + 1)],
                                rhs=hid_sb[:, 2 * u : 2 * u + 2,
                                           512 * nh : 512 * (nh + 1)],
                                start=(u == 0),
                                stop=(u == 1),
                                perf_mode=PM.DoubleRow,
                            )
                    else:
                        loops = (
                            [(kc, nh) for kc in range(4) for nh in range(2)]
                            if cfg["reorder"]
                            else [(kc, nh) for nh in range(2) for kc in range(4)]
                        )
                        for kc, nh in loops:
                            nc.tensor.matmul(
                                ps[:, 512 * nh : 512 * (nh + 1)],
                                lhsT=wp_sb[:, kc, 128 * m : 128 * (m + 1)],
                                rhs=hid_sb[:, kc, 512 * nh : 512 * (nh + 1)],
                                start=(kc == 0),
                                stop=(kc == 3),
                            )
                    def drain_col(c0, c1):
                        ob = otp.tile([128, c1 - c0], odt, tag="ob")
                        if cfg["bp_fuse"]:
                            nc.vector.scalar_tensor_tensor(
                                out=ob,
                                in0=ps[:, c0:c1],
                                scalar=bp_sb[:, m : m + 1],
                                in1=x_sb[:, m, c0:c1],
                                op0=OP.add,
                                op1=OP.add,
                            )
                        else:
                            nc.vector.tensor_add(
                                out=ob, in0=ps[:, c0:c1], in1=x_sb[:, m, c0:c1]
                            )
                        nc.sync.dma_start(
                            out=outd[128 * m : 128 * (m + 1), c0:c1], in_=ob
                        )

                    if cfg["drain_split"]:
                        drain_col(0, 512)
                        drain_col(512, 1024)
                    else:
                        drain_col(0, 1024)

            if loop_n:
                ws = emit_weight_loads()
                with tc.For_i(0, loop_n, 1):
                    _emit_body(ws)
            else:
                x_sb = alloc_x()
                ws = emit_weight_loads()
                _emit_body(ws, x_pre=x_sb)

    _split_waits(nc)
    return nc


_NC = None


def _get_nc():
    global _NC
    if _NC is None:
        _install_compat()
        _NC = build_nc()
    return _NC


def _pack_rows(W, o):
    # [C, o] -> [128, 4*o]: row c = a*128 + p  ->  partition p, chunk a
    return np.ascontiguousarray(
        W.reshape(4, 128, o).transpose(1, 0, 2).reshape(128, 4 * o)
    )


def _host_prep(x, gamma, beta, w_qkv, b_qkv, w_proj, b_proj):
    x = np.asarray(x, np.float32)
    gamma = np.asarray(gamma, np.float32)
    beta = np.asarray(beta, np.float32)
    w_qkv = np.asarray(w_qkv, np.float32)
    b_qkv = np.asarray(b_qkv, np.float32)
    w_proj = np.asarray(w_proj, np.float32)
    b_proj = np.asarray(b_proj, np.float32)

    s2 = 1.0 / np.sqrt(CH)  # attention scale applied to q AND k => s^2 on q
    Wg = w_qkv * gamma[None, :]
    bb = w_qkv @ beta + b_qkv
    Wg = Wg.copy()
    Wg[0:C] *= s2
    bb = bb.copy()
    bb[0:C] *= s2

    WDT = F8 if DEFAULT_CFG["fp8"] else BF16
    shared = {
        "wqk": _pack_rows(np.ascontiguousarray(Wg[0 : 2 * C].T), 2 * C).astype(WDT),
        "wv": _pack_rows(np.ascontiguousarray(Wg[2 * C : 3 * C].T), C).astype(WDT),
        "wp": _pack_rows(np.ascontiguousarray(w_proj.T), C).astype(WDT),
        "bqk": np.ascontiguousarray(bb[0 : 2 * C].reshape(8, 128).T).astype(
            np.float32
        ),
        "bvb": np.broadcast_to(bb[2 * C : 3 * C].reshape(1, C), (128, C)).astype(
            BF16
        ),
        "bp": np.ascontiguousarray(b_proj.reshape(4, 128).T).astype(np.float32),
        "ind": (np.arange(128)[:, None] // 16 == np.arange(8)[None, :]).astype(
            np.float32
        ),
        "indT": (np.arange(128)[None, :] // 16 == np.arange(8)[:, None]).astype(
            np.float32
        ),
    }
    in_maps = []
    for b in range(N_CORES):
        m = dict(shared)
        m["x"] = _pack_rows(x[b].reshape(C, L), L).astype(BF16)
        in_maps.append(m)
    return in_maps


def run_spmd(in_maps, trace=False):
    from concourse.bass_utils import run_bass_kernel_spmd

    nc = _get_nc()
    return run_bass_kernel_spmd(
        nc, in_maps, core_ids=list(range(N_CORES)), trace=trace
    )


def kernel(x, gamma, beta, w_qkv, b_qkv, w_proj, b_proj):
    _install_compat()
    in_maps = _host_prep(x, gamma, beta, w_qkv, b_qkv, w_proj, b_proj)
    res = run_spmd(in_maps, trace=False)
    out = np.stack(
        [res.results[c]["out"].reshape(C, 32, 32) for c in range(N_CORES)]
    ).astype(np.float32)
    return out


# revision 73
# speedup vs baseline: 1.5720x; 1.2861x over previous
"""AttentionBlock (GroupNorm -> qkv -> 8-head attention -> proj -> residual)
as a distributed Bass/Tile kernel on 8 TRN2 NeuronCores.

Sharding: pure data-parallel over batch B=8 -> one batch element per core,
zero collectives. Each core computes its whole attention block.

Per-core algorithm (C=512, L=1024, NH=8, ch=64, G=32 groups):
  - x shipped bf16, host-pre-transposed to [128, 4*L] (fat contiguous
    per-partition descriptors), issued 2 dma_starts per chunk BEFORE the
    weight DMAs (each dma_start maps to one ~26GB/s HW-DGE queue).
  - GroupNorm stats: chunks 0-2 via DVE bn_stats, chunk 3 via ScalarE
    Copy/Square activations with accum_out (parallel engines; square sits
    in the same ACT table as ln/exp); tiny PE matmuls reduce channel stats
    to group stats (16 channels/group) and broadcast back. gamma/beta
    folded into the qkv weights host-side, attention scale (ch^-1/4 on q
    and k) folded into the q weights as 1/sqrt(ch); GN apply split DVE /
    ScalarE (activation with per-partition scale+bias).
  - qkv / v / AV / proj matmuls in fp8 e4m3 with MatmulPerfMode.DoubleRow
    (2 k-tiles per pass, ~2.2x measured PE throughput); scores in bf16
    (64-deep contraction cannot DoubleRow). q,k drained to bf16; xn, vT,
    softmax probabilities and hid kept in fp8.
  - scores computed TRANSPOSED: sT[s, t] = k^T q (lhsT=k, rhs=q); the two
    heads of a pair sit at partitions 0:64 / 64:128. v produced directly
    transposed by swapping matmul operands -- no on-chip transposes.
  - softmax computed SHIFTED: p' = exp(s - 2.5) (cancels in p'/D'), which
    keeps the fp8 exp in range. Exp split ScalarE table Exp (f8 out) /
    DVE Schraudolph writing fp8 BITS via uint8 bitcast (float->uint
    saturates negatives to 0 = fp8 +0 on deep underflow). GPSIMD cannot
    read PSUM on this HW, so only these two engines can run the exp.
  - AV lhsT is [v | ones*64], so rows 64:128 of the AV PSUM tile hold the
    softmax denominator D replicated across 64 partitions; division is
    ScalarE exp(-ln(D)) (Ln/Exp share one ACT table with the softmax Exp,
    no table switch) + one DVE multiply.
  - software-pipelined schedule: scores(0)/(1) emitted inside the qkv
    phase so the exp stream overlaps qkv; scores(3) halves interleaved
    with AV(2) tiles so the PE always has independent DoubleRow work
    queued when scores stall on the exp/PSUM rotation; b_proj folded into
    the proj drain (scalar_tensor_tensor).
  - PE warm-up drip: dummy matmuls during the DMA/GN head keep the Tensor
    engine p-state ramped (2.4GHz warm vs 1.2GHz after idle).
  - proj drained per 512-col half, output written bf16 (host upcasts).
"""

import sys
import types

import numpy as np
import ml_dtypes

BF16 = ml_dtypes.bfloat16
F8 = ml_dtypes.float8_e4m3

C = 512
L = 1024
NH = 8
CH = 64
G = 32
EPS = 1e-5
N_CORES = 8


# ---------------------------------------------------------------------------
# Environment compat (inlined so kernel.py is self-contained)
# ---------------------------------------------------------------------------
def _install_compat():
    # 1) NTFF profiling hook shim (image's antenv stub lacks axon_hooks).
    try:
        from antenv.axon_hooks import get_axon_ntff_profile_hook  # noqa: F401
    except ImportError:
        try:
            import antenv
            from trn_agent_boot.trn_boot import _ntff_profile_via_ctypes

            m = types.ModuleType("antenv.axon_hooks")
            m._hook = None
            m.set_axon_ntff_profile_hook = lambda h: setattr(m, "_hook", h)
            m.get_axon_ntff_profile_hook = lambda: m._hook
            sys.modules["antenv.axon_hooks"] = m
            antenv.axon_hooks = m
            m.set_axon_ntff_profile_hook(
                _ntff_profile_via_ctypes("/opt/axon/libaxon_pjrt.so")
            )
        except Exception:
            pass

    # 2) gpsimd.sem_clear over a wide semaphore range exceeds this walrus
    #    build's ISA payload limit ("ISA wrong length"); chunk the clears.
    import concourse.bass as bass

    if not getattr(bass.Bass.clear_and_free_semaphores, "_chunk_patch", False):
        _orig_clear = bass.Bass.clear_and_free_semaphores

        def _chunked_clear(self, sems, _orig=_orig_clear):
            sems = list(sems)
            for i in range(0, len(sems), 4):
                _orig(self, sems[i : i + 4])

        _chunked_clear._chunk_patch = True
        bass.Bass.clear_and_free_semaphores = _chunked_clear


def _split_waits(nc):
    """This walrus build accepts at most ONE semaphore wait per instruction;
    Tile emits up to 2 (and the closing drain more). Split the extras into
    standalone EVENT_SEM instructions inserted just before, on the same
    engine, which is semantically identical (same-engine program order)."""
    from concourse import mybir

    nid = 0
    for blk in nc.m.functions[0].blocks:
        new_list = []
        for inst in blk.instructions:
            si = inst.sync_info
            if si and si.on_wait and len(si.on_wait) > 1:
                waits = list(si.on_wait)
                si.on_wait = waits[-1:]
                for w in waits[:-1]:
                    nid += 1
                    ev = mybir.InstEventSemaphore(
                        name=f"WSPLIT-{nid}", ins=[], outs=[]
                    )
                    ev.engine = inst.engine
                    ev.sync_info = mybir.SyncInfo(on_wait=[w], on_update=[])
                    nc.register_instruction(ev, overwrite=True)
                    new_list.append(ev)
            new_list.append(inst)
        blk.instructions[:] = new_list


# ---------------------------------------------------------------------------
# Bass graph
# ---------------------------------------------------------------------------
# Schraudolph exp constants (bf16 bit-space)
A16 = 184.6649652337873
B16 = 16250.37
# Schraudolph exp constants (fp8 e4m3 bit-space); the softmax is computed
# shifted, p' = exp(s - CSHIFT), which cancels in p'/sum(p') -- keeps the
# fp8 exp under the e4m3 finite max (240) for |s| <= ~8 and keeps typical
# probabilities in the normal range.
A8 = 11.541560327111707
B8 = 55.65
CSHIFT = 2.5

# exp engine maps: per (st, hh) -> "dve", default ScalarE. (GPSIMD cannot
# read PSUM on this HW, so only ScalarE/DVE can run the softmax exp.)
# j=0..2 run inside the qkv phase (ScalarE-heavy); j=3 runs in the AV tail
# where DVE must carry the division too.
EXP_EARLY = {  # j = 0, 1, 2
    (1, 0): "dve", (3, 1): "dve", (5, 0): "dve", (6, 1): "dve", (7, 0): "dve",
}
EXP_LATE = {  # j = 3
    (1, 0): "dve", (3, 1): "dve", (5, 0): "dve", (6, 1): "dve",
    (0, 1): "dve", (2, 0): "dve", (4, 1): "dve", (7, 0): "dve",
}

DEFAULT_CFG = dict(
    reorder=True,        # kc-outer/nh-inner in qkv/proj
    no_exp=False,        # probe: skip softmax exp (timing only)
    no_div=False,        # probe: skip division chain
    no_gn=False,         # probe: skip GroupNorm stats chain (memset scales)
    psum3=True,          # scores tag A bufs=3; av split by nh to [128, 512]
    xdbuf=True,          # x_sb in a bufs=2 pool (loop mode overlap)
    swpipe=True,         # software-pipeline attention: scores(j+1) before AV(j)
    ptbufs=5,            # pt pool buffers
    early=True,          # scores(0)/(1) inside the qkv phase
    div="lnexp",         # "lnexp" (ScalarE exp(-ln(D))) | "recip" (DVE, slow)
                         # ("approx" custom-DVE recip hits this walrus
                         # build's "ISA wrong length" limit)
    warm=(12, 2, 2),     # PE warmup matmul counts (pre-GN, mid-GN, post-GN);
                         # w1=20 measured worse (warmups sit ahead of the
                         # GN group matmul in PE order and delay it); w2/w3
                         # sit on the PE critical path so keep them tiny
    gn_pool=0,           # GN-apply chunks on Pool (Pool is ~20x slower for
                         # tensor ops on this HW -- memsets only)
    bp_fuse=True,        # fold x += b_proj into the proj drain (one
                         # scalar_tensor_tensor op: (ps + bp) + x)
    drain_split=True,    # proj drain + out DMA per 512-col half
    obf16=True,          # output DRAM tensor in bf16 (host upcasts)
    early3=False,        # scores(0..2) inside the qkv phase (with the
                         # DVE-heavy exp maps, 2 early scores measured best)
    v_ilv=False,         # interleave scores(2) halves with the v phase
                         # (measured WORSE: s2's DVE exps block the v
                         # drains queued behind them -- priority inversion)
    x1dma=True,          # x as one 3D-AP dma (wide queue fan-out)
    fp8=True,            # qkv/v/AV/proj matmuls in fp8 e4m3 DoubleRow
                         # (2.2x PE throughput); scores stay bf16
)


def build_nc(loop_n=None, cfg=None):
    import concourse.bass as bass
    import concourse.tile as tile
    from concourse import mybir

    cfg = {**DEFAULT_CFG, **(cfg or {})}
    i16 = mybir.dt.int16
    u8 = mybir.dt.uint8
    f32 = mybir.dt.float32
    bf = mybir.dt.bfloat16
    f8 = mybir.dt.float8e4
    PM = mybir.MatmulPerfMode
    AF = mybir.ActivationFunctionType
    OP = mybir.AluOpType
    FP8 = cfg["fp8"]
    adt = f8 if FP8 else bf  # activation dtype for xn/v/pts/hid

    nc = bass.Bass(trn_type="TRN2")
    xd = nc.declare_dram_parameter("x", [128, 4 * L], bf, isOutput=False)
    wqkd = nc.declare_dram_parameter("wqk", [128, 8 * C], adt, isOutput=False)
    wvd = nc.declare_dram_parameter("wv", [128, 4 * C], adt, isOutput=False)
    wpd = nc.declare_dram_parameter("wp", [128, 4 * C], adt, isOutput=False)
    bqkd = nc.declare_dram_parameter("bqk", [128, 8], f32, isOutput=False)
    bvd = nc.declare_dram_parameter("bvb", [128, C], bf, isOutput=False)
    bpd = nc.declare_dram_parameter("bp", [128, 4], f32, isOutput=False)
    indd = nc.declare_dram_parameter("ind", [128, 8], f32, isOutput=False)
    indTd = nc.declare_dram_parameter("indT", [8, 128], f32, isOutput=False)
    odt = bf if cfg["obf16"] else f32
    outd = nc.declare_dram_parameter("out", [C, L], odt, isOutput=True)

    with tile.TileContext(nc) as tc:
        with (
            tc.tile_pool(name="cst", bufs=1) as cst,
            tc.tile_pool(name="act", bufs=1) as actp,
            tc.tile_pool(name="xbp", bufs=2) as xbp,
            tc.tile_pool(name="ptp", bufs=cfg["ptbufs"]) as ptp,
            tc.tile_pool(name="dnp", bufs=4) as dnp,
            tc.tile_pool(name="otp", bufs=8) as otp,
            tc.tile_pool(name="psp", bufs=1, space="PSUM") as psp,
        ):
            # warmup constant (memset on Pool: cheap and dep-free)
            wrm = cst.tile([128, 512], bf)
            nc.gpsimd.memset(wrm, 0.125)
            # per-partition -CSHIFT bias tile for the shifted fp8 exp
            csh = cst.tile([128, 1], f32)
            nc.gpsimd.memset(csh, -CSHIFT)
            # dependency-free dummy activation: forces the ACT_TABLE_LOAD
            # (1.3us) to run at engine boot instead of lazily before the
            # first real activation (which waits on the x DMA)
            twarm = cst.tile([128, 1], f32)
            nc.scalar.activation(out=twarm, in_=csh, func=AF.Exp)

            def emit_warm(n):
                for _ in range(n):
                    wp_ = psp.tile([128, 512], f32, tag="B", bufs=2)
                    nc.tensor.matmul(
                        wp_, lhsT=wrm[:, 0:128], rhs=wrm, start=True, stop=True
                    )

            def alloc_x():
                xpool = xbp if cfg["xdbuf"] else actp
                x_sb = xpool.tile([128, 4, L], bf)
                xr = xd[:, :].rearrange("p (a o) -> p a o", o=L)
                if cfg["x1dma"]:
                    # one 3D-AP dma (512 descriptors): 3D APs fan out across
                    # ~10 HW-DGE queues (observed on the weight loads), so
                    # all chunks land together and early; splitting into two
                    # 3D dmas measured WORSE (ring overhead, no extra fan)
                    nc.sync.dma_start(out=x_sb, in_=xr)
                else:
                    for t in range(4):
                        nc.sync.dma_start(
                            out=x_sb[0:64, t, :], in_=xr[0:64, t, :]
                        )
                        nc.sync.dma_start(
                            out=x_sb[64:128, t, :], in_=xr[64:128, t, :]
                        )
                return x_sb

            def emit_weight_loads():
                wqk_sb = cst.tile([128, 4, 2 * C], adt)
                nc.sync.dma_start(
                    out=wqk_sb,
                    in_=wqkd[:, :].rearrange("p (a o) -> p a o", o=2 * C),
                )
                wv_sb = cst.tile([128, 4, C], adt)
                nc.sync.dma_start(
                    out=wv_sb, in_=wvd[:, :].rearrange("p (a o) -> p a o", o=C)
                )
                wp_sb = cst.tile([128, 4, C], adt)
                nc.sync.dma_start(
                    out=wp_sb, in_=wpd[:, :].rearrange("p (a o) -> p a o", o=C)
                )
                bqk_sb = cst.tile([128, 8], f32)
                nc.sync.dma_start(out=bqk_sb, in_=bqkd[:, :])
                bvb_sb = cst.tile([128, C], bf)
                nc.sync.dma_start(out=bvb_sb, in_=bvd[:, :])
                bp_sb = cst.tile([128, 4], f32)
                nc.sync.dma_start(out=bp_sb, in_=bpd[:, :])
                ind_sb = cst.tile([128, 8], f32)
                nc.sync.dma_start(out=ind_sb, in_=indd[:, :])
                indT_sb = cst.tile([8, 128], f32)
                nc.sync.dma_start(out=indT_sb, in_=indTd[:, :])
                eps8 = cst.tile([8, 1], f32)
                nc.vector.memset(eps8, EPS)
                return (wqk_sb, wv_sb, wp_sb, bqk_sb, bvb_sb, bp_sb, ind_sb,
                        indT_sb, eps8)

            def _emit_body(ws, x_pre=None):
                (wqk_sb, wv_sb, wp_sb, bqk_sb, bvb_sb, bp_sb, ind_sb,
                 indT_sb, eps8) = ws
                x_sb = x_pre if x_pre is not None else alloc_x()

                xn_sb = actp.tile([128, 4, L], adt)
                q_sb = actp.tile([128, 4, L], bf)
                k_sb = actp.tile([128, 4, L], bf)
                VW = 2 * CH
                vT_sb = actp.tile([128, 8, NH, VW], adt)
                hid_sb = actp.tile([128, 4, L], adt)
                sc_sb = actp.tile([128, 8], f32)
                if cfg["no_div"]:
                    for t in range(4):
                        nc.vector.memset(hid_sb[:, t, :], 0.001)

                if cfg["warm"][0]:
                    emit_warm(cfg["warm"][0])

                # ---- GroupNorm statistics. stats4 cols (2t, 2t+1) hold
                # (mean, E[x^2]) per channel: chunks 0-2 via DVE bn_stats,
                # chunk 3 via ScalarE Copy/Square with accum_out (the engines
                # run in parallel; square shares the ln/exp ACT table)
                st6 = actp.tile([128, 3, 2, 6], f32)
                mv = actp.tile([128, 3, 2], f32)
                stats4 = actp.tile([128, 8], f32)
                sqs = actp.tile([128, L], bf)
                if cfg["no_gn"]:
                    nc.vector.memset(sc_sb, 1.0)
                # ScalarE takes chunk 0 (the x chunks land ~1.2us apart in
                # DMA issue order, so chunk 0 is ready first and the slower
                # 2-pass ScalarE pipeline gets the head start); DVE bn_stats
                # take chunks 1-3 as they land
                if not cfg["no_gn"]:
                    nc.scalar.activation(
                        out=sqs, in_=x_sb[:, 0, :], func=AF.Copy,
                        scale=1.0 / L, accum_out=stats4[:, 0:1],
                    )
                    nc.scalar.activation(
                        out=sqs, in_=x_sb[:, 0, :], func=AF.Square,
                        scale=1.0 / 32.0, accum_out=stats4[:, 1:2],
                    )
                for t in [] if cfg["no_gn"] else range(1, 4):
                    for s in range(2):
                        nc.vector.bn_stats(
                            out=st6[:, t - 1, s, :],
                            in_=x_sb[:, t, 512 * s : 512 * (s + 1)],
                        )
                    nc.vector.bn_aggr(
                        out=mv[:, t - 1, :], in_=st6[:, t - 1, :, :]
                    )
                if not cfg["no_gn"]:
                    s4 = stats4.rearrange("p (t s) -> p t s", s=2)
                    nc.vector.tensor_copy(out=s4[:, 1:4, 0], in_=mv[:, :, 0])
                    nc.vector.tensor_mul(
                        out=s4[:, 1:4, 1], in0=mv[:, :, 0], in1=mv[:, :, 0]
                    )
                    nc.vector.tensor_add(
                        out=s4[:, 1:4, 1], in0=s4[:, 1:4, 1], in1=mv[:, :, 1]
                    )
                    gmm = psp.tile([8, 8], f32, tag="B", bufs=2)
                    nc.tensor.matmul(
                        gmm, lhsT=ind_sb, rhs=stats4, start=True, stop=True
                    )
                    if cfg["warm"][1]:
                        emit_warm(cfg["warm"][1])
                    gm = actp.tile([8, 8], f32)
                    nc.vector.tensor_scalar_mul(out=gm, in0=gmm, scalar1=1.0 / 16.0)
                    gmr = gm.rearrange("g (t s) -> g t s", s=2)
                    msq = actp.tile([8, 4], f32)
                    nc.vector.tensor_mul(out=msq, in0=gmr[:, :, 0], in1=gmr[:, :, 0])
                    gv = actp.tile([8, 4], f32)
                    nc.vector.tensor_tensor(
                        out=gv, in0=gmr[:, :, 1], in1=msq, op=OP.subtract
                    )
                    # rsqrt(v + eps) = exp(-0.5 * ln(v + eps)): Log and Exp
                    # share one ACT table set, so the softmax Exp later needs
                    # no table switch.
                    sd = actp.tile([8, 4], f32)
                    nc.scalar.activation(out=sd, in_=gv, func=AF.Ln, bias=eps8, scale=1.0)
                    inv8 = actp.tile([8, 4], f32)
                    nc.scalar.activation(out=inv8, in_=sd, func=AF.Exp, scale=-0.5)
                    sh8 = actp.tile([8, 4], f32)
                    nc.vector.tensor_mul(out=sh8, in0=gmr[:, :, 0], in1=inv8)
                    nc.vector.tensor_scalar_mul(out=sh8, in0=sh8, scalar1=-1.0)
                    gs = actp.tile([8, 8], f32)
                    gsr = gs.rearrange("g (t s) -> g t s", s=2)
                    nc.vector.tensor_copy(out=gsr[:, :, 0], in_=inv8)
                    nc.vector.tensor_copy(out=gsr[:, :, 1], in_=sh8)
                    nb = psp.tile([128, 8], f32, tag="B", bufs=2)
                    nc.tensor.matmul(nb, lhsT=indT_sb, rhs=gs, start=True, stop=True)
                    nc.vector.tensor_copy(out=sc_sb, in_=nb)
                if cfg["warm"][2]:
                    emit_warm(cfg["warm"][2])
                for t in range(4):
                    # split applies across DVE and ScalarE (per-partition
                    # scale/bias activation) so they finish in half the time
                    if t >= 2:
                        nc.scalar.activation(
                            out=xn_sb[:, t, :],
                            in_=x_sb[:, t, :],
                            func=AF.Identity,
                            scale=sc_sb[:, 2 * t : 2 * t + 1],
                            bias=sc_sb[:, 2 * t + 1 : 2 * t + 2],
                        )
                    else:
                        nc.vector.tensor_scalar(
                            out=xn_sb[:, t, :],
                            in0=x_sb[:, t, :],
                            scalar1=sc_sb[:, 2 * t : 2 * t + 1],
                            scalar2=sc_sb[:, 2 * t + 1 : 2 * t + 2],
                            op0=OP.mult,
                            op1=OP.add,
                        )

                if not cfg["bp_fuse"]:
                    # residual base: x + b_proj (in place)
                    for m in range(4):
                        nc.vector.tensor_scalar_add(
                            out=x_sb[:, m, :], in0=x_sb[:, m, :],
                            scalar1=bp_sb[:, m : m + 1],
                        )

                # ---- qkv: q,k in natural [c, l] layout
                def emit_qkv_m(m):
                    dst = q_sb if m < 4 else k_sb
                    # nh-split 1-bank tag-B tiles so the scores pss (tag A)
                    # rotation never blocks qkv
                    for nh in range(2):
                        ps = psp.tile([128, 512], f32, tag="B", bufs=2)
                        if FP8:
                            for u in range(2):
                                nc.tensor.matmul(
                                    ps,
                                    lhsT=wqk_sb[:, 2 * u : 2 * u + 2,
                                                128 * m : 128 * (m + 1)],
                                    rhs=xn_sb[:, 2 * u : 2 * u + 2,
                                              512 * nh : 512 * (nh + 1)],
                                    start=(u == 0),
                                    stop=(u == 1),
                                    perf_mode=PM.DoubleRow,
                                )
                        else:
                            for kc in range(4):
                                nc.tensor.matmul(
                                    ps,
                                    lhsT=wqk_sb[:, kc, 128 * m : 128 * (m + 1)],
                                    rhs=xn_sb[:, kc, 512 * nh : 512 * (nh + 1)],
                                    start=(kc == 0),
                                    stop=(kc == 3),
                                )
                        nc.vector.tensor_scalar_add(
                            out=dst[:, m % 4, 512 * nh : 512 * (nh + 1)],
                            in0=ps, scalar1=bqk_sb[:, m : m + 1],
                        )

                def emit_v_phase(lts=range(8), ones=True):
                    # v, produced directly transposed: vT[l, c] + ones cols
                    # (ones blocks on Pool: off the critical path)
                    if ones:
                        for lt in range(8):
                            nc.gpsimd.memset(vT_sb[:, lt, :, CH:VW], 1.0)
                    for lt in lts:
                        ps = psp.tile([128, C], f32, tag="B", bufs=2)
                        if FP8:
                            for u in range(2):
                                nc.tensor.matmul(
                                    ps,
                                    lhsT=xn_sb[:, 2 * u : 2 * u + 2,
                                               128 * lt : 128 * (lt + 1)],
                                    rhs=wv_sb[:, 2 * u : 2 * u + 2, :],
                                    start=(u == 0),
                                    stop=(u == 1),
                                    perf_mode=PM.DoubleRow,
                                )
                        else:
                            for kc in range(4):
                                nc.tensor.matmul(
                                    ps,
                                    lhsT=xn_sb[:, kc, 128 * lt : 128 * (lt + 1)],
                                    rhs=wv_sb[:, kc, :],
                                    start=(kc == 0),
                                    stop=(kc == 3),
                                )
                        nc.vector.tensor_tensor(
                            out=vT_sb[:, lt, :, 0:CH],
                            in0=ps.rearrange("p (h c) -> p h c", h=NH),
                            in1=bvb_sb.rearrange("p (h c) -> p h c", h=NH),
                            op=OP.add,
                        )

                # ---- attention, head pairs (2j at partitions 0:64,
                #      2j+1 at 64:128)
                def emit_scores(j, pts=None, sts=None, emap=None):
                    m = j
                    n_early = 3 if cfg["early3"] else 2
                    if emap is None:
                        emap = (
                            EXP_EARLY if (cfg["early"] and j < n_early)
                            else EXP_LATE
                        )
                    if pts is None:
                        pts = [
                            ptp.tile([128, 8, L], adt, tag="pt", name=f"pt{j}_0"),
                            ptp.tile([128, 8, L], adt, tag="pt", name=f"pt{j}_1"),
                        ]
                    if cfg["no_exp"] and sts is None:
                        for st in range(8):
                            nc.vector.memset(pts[0][:, st, :], 0.001)
                            nc.vector.memset(pts[1][:, st, :], 0.001)
                    for st in (range(8) if sts is None else sts):
                        abufs = 3 if cfg["psum3"] else 2
                        pss = [
                            psp.tile([128, L], f32, tag="A", bufs=abufs,
                                     name=f"qkt{j}_{st}_0"),
                            psp.tile([128, L], f32, tag="A", bufs=abufs,
                                     name=f"qkt{j}_{st}_1"),
                        ]
                        # sequential per-head emission measured faster on HW
                        # than interleaving the pair's matmuls
                        for hh in range(2):
                            for nh in range(2):
                                po = 64 * hh
                                nc.tensor.matmul(
                                    pss[hh][:, 512 * nh : 512 * (nh + 1)],
                                    lhsT=k_sb[po : po + 64, m, 128 * st : 128 * (st + 1)],
                                    rhs=q_sb[po : po + 64, m, 512 * nh : 512 * (nh + 1)],
                                    start=True,
                                    stop=True,
                                )
                        for hh in [] if cfg["no_exp"] else range(2):
                            eng = emap.get((st, hh))
                            if eng == "dve":
                                # u8 bitcast: float->uint saturates negatives
                                # to 0, so deep-underflow exps land on f8 +0
                                # instead of wrapping to garbage
                                iv = pts[hh][:, st, :].bitcast(
                                    u8 if FP8 else i16
                                )
                                sa, sb = (A8, B8 - A8 * CSHIFT) if FP8 else (
                                    A16, B16
                                )
                                with nc.allow_low_precision(
                                    reason="Schraudolph exp ripple ok for softmax"
                                ):
                                    nc.vector.tensor_scalar(
                                        out=iv, in0=pss[hh],
                                        scalar1=sa, scalar2=sb,
                                        op0=OP.mult, op1=OP.add,
                                    )
                            else:
                                nc.scalar.activation(
                                    out=pts[hh][:, st, :], in_=pss[hh],
                                    func=AF.Exp,
                                    bias=csh if FP8 else 0.0,
                                )
                    return pts

                def emit_div(av, j, hh, nh, m):
                    if cfg["no_div"]:
                        return
                    dst = hid_sb[
                        64 * hh : 64 * hh + CH, m, 512 * nh : 512 * (nh + 1)
                    ]
                    if cfg["div"] == "approx":
                        dinv = dnp.tile([CH, 512], f32, tag="dsb")
                        nc.vector.reciprocal_approx_fast(
                            out=dinv, in_=av[CH:128, :]
                        )
                    elif cfg["div"] == "lnexp":
                        lnb = dnp.tile([CH, 512], f32, tag="lnb")
                        nc.scalar.activation(
                            out=lnb, in_=av[CH:128, :], func=AF.Ln
                        )
                        dinv = dnp.tile([CH, 512], bf, tag="dsb")
                        nc.scalar.activation(
                            out=dinv, in_=lnb, func=AF.Exp, scale=-1.0
                        )
                    else:
                        dinv = dnp.tile([CH, 512], bf, tag="dsb")
                        with nc.allow_low_precision(
                            reason="softmax 1/D in bf16 is within tolerance"
                        ):
                            nc.vector.reciprocal(out=dinv, in_=av[CH:128, :])
                    nc.vector.tensor_mul(out=dst, in0=av[0:CH, :], in1=dinv)

                def emit_av(j, pts, hhs=(0, 1)):
                    m = j
                    # finer AV granularity: one [128, 512] PSUM tile per
                    # (hh, nh); the division of one tile overlaps the AV
                    # matmuls of the next
                    for hh in hhs:
                        for nh in range(2):
                            av = psp.tile(
                                [128, 512], f32, tag="B", bufs=2,
                                name=f"av{j}_{hh}_{nh}",
                            )
                            if FP8:
                                for u in range(4):
                                    nc.tensor.matmul(
                                        av,
                                        lhsT=vT_sb[:, 2 * u : 2 * u + 2,
                                                   2 * j + hh, :],
                                        rhs=pts[hh][:, 2 * u : 2 * u + 2,
                                                    512 * nh : 512 * (nh + 1)],
                                        start=(u == 0),
                                        stop=(u == 3),
                                        perf_mode=PM.DoubleRow,
                                    )
                            else:
                                for st in range(8):
                                    nc.tensor.matmul(
                                        av,
                                        lhsT=vT_sb[:, st, 2 * j + hh, :],
                                        rhs=pts[hh][:, st,
                                                    512 * nh : 512 * (nh + 1)],
                                        start=(st == 0),
                                        stop=(st == 7),
                                    )
                            emit_div(av, j, hh, nh, m)

                if cfg["early"] and cfg["early3"]:
                    # scores(0..2) emitted inside the qkv phase: the exp
                    # stream overlaps the qkv matmuls; only scores(3)'s exps
                    # remain in the AV tail
                    emit_qkv_m(0)
                    emit_qkv_m(4)
                    pts0 = emit_scores(0)
                    emit_qkv_m(1)
                    emit_qkv_m(5)
                    pts1 = emit_scores(1)
                    emit_qkv_m(2)
                    emit_qkv_m(6)
                    pts2 = emit_scores(2)
                    emit_qkv_m(3)
                    emit_qkv_m(7)
                    emit_v_phase()
                    emit_av(0, pts0)
                    pts3 = emit_scores(3)
                    emit_av(1, pts1)
                    emit_av(2, pts2)
                    emit_av(3, pts3)
                elif cfg["early"]:
                    emit_qkv_m(0)
                    emit_qkv_m(4)
                    pts0 = emit_scores(0)
                    emit_qkv_m(1)
                    emit_qkv_m(5)
                    pts1 = emit_scores(1)
                    for m_ in (2, 6, 3, 7):
                        emit_qkv_m(m_)
                    if cfg["v_ilv"]:
                        # interleave scores(2) halves with the v phase so
                        # its exp stream starts during the v matmuls
                        pts2 = emit_scores(2, sts=range(0, 4), emap=EXP_EARLY)
                        emit_v_phase(lts=range(0, 4))
                        emit_scores(2, pts=pts2, sts=range(4, 8),
                                    emap=EXP_EARLY)
                        emit_v_phase(lts=range(4, 8), ones=False)
                        emit_av(0, pts0)
                        emit_av(1, pts1)
                    else:
                        emit_v_phase()
                        emit_av(0, pts0)
                        pts2 = emit_scores(2)
                        emit_av(1, pts1)
                    # interleave scores(3) halves with av(2) tiles so the PE
                    # has independent DR work queued when the scores matmuls
                    # stall on the tag-A/exp rotation
                    pts3 = emit_scores(3, sts=range(0, 4))
                    emit_av(2, pts2, hhs=(0,))
                    emit_scores(3, pts=pts3, sts=range(4, 8))
                    emit_av(2, pts2, hhs=(1,))
                    emit_av(3, pts3)
                elif cfg["swpipe"]:
                    for m_ in (0, 4, 1, 5, 2, 6, 3, 7):
                        emit_qkv_m(m_)
                    emit_v_phase()
                    ptss = {0: emit_scores(0), 1: emit_scores(1)}
                    emit_av(0, ptss.pop(0))
                    ptss[2] = emit_scores(2)
                    emit_av(1, ptss.pop(1))
                    ptss[3] = emit_scores(3)
                    emit_av(2, ptss.pop(2))
                    emit_av(3, ptss.pop(3))
                else:
                    for m_ in (0, 4, 1, 5, 2, 6, 3, 7):
                        emit_qkv_m(m_)
                    emit_v_phase()
                    for j in range(4):
                        emit_av(j, emit_scores(j))

                # ---- proj + residual (residual base x+b_p precomputed)
                for m in range(4):
                    ps = psp.tile(
                        [128, L], f32, tag="A", bufs=3 if cfg["psum3"] else 2,
                        name=f"proj{m}",
                    )
                    if FP8:
                        for u, nh in [(u, nh) for u in range(2) for nh in range(2)]:
                            nc.tensor.matmul(
                                ps[:, 512 * nh : 512 * (nh + 1)],
                                lhsT=wp_sb[:, 2 * u : 2 * u + 2,
                                           128 * m : 128 * (m + 1)],
                                rhs=hid_sb[:, 2 * u : 2 * u + 2,
                                           512 * nh : 512 * (nh + 1)],
                                start=(u == 0),
                                stop=(u == 1),
                                perf_mode=PM.DoubleRow,
                            )
                    else:
                        loops = (
                            [(kc, nh) for kc in range(4) for nh in range(2)]
                            if cfg["reorder"]
                            else [(kc, nh) for nh in range(2) for kc in range(4)]
                        )
                        for kc, nh in loops:
                            nc.tensor.matmul(
                                ps[:, 512 * nh : 512 * (nh + 1)],
                                lhsT=wp_sb[:, kc, 128 * m : 128 * (m + 1)],
                                rhs=hid_sb[:, kc, 512 * nh : 512 * (nh + 1)],
                                start=(kc == 0),
                                stop=(kc == 3),
                            )
                    def drain_col(c0, c1):
                        ob = otp.tile([128, c1 - c0], odt, tag="ob")
                        if cfg["bp_fuse"]:
                            nc.vector.scalar_tensor_tensor(
                                out=ob,
                                in0=ps[:, c0:c1],
                                scalar=bp_sb[:, m : m + 1],
                                in1=x_sb[:, m, c0:c1],
                                op0=OP.add,
                                op1=OP.add,
                            )
                        else:
                            nc.vector.tensor_add(
                                out=ob, in0=ps[:, c0:c1], in1=x_sb[:, m, c0:c1]
                            )
                        # NOTE: 3D-AP views on these writes measured WORSE
                        # (o=128 and o=256 both +3us): the sub-1KB
                        # descriptors cost more than the queue fan gains
                        nc.sync.dma_start(
                            out=outd[128 * m : 128 * (m + 1), c0:c1], in_=ob
                        )

                    if cfg["drain_split"]:
                        drain_col(0, 512)
                        drain_col(512, 1024)
                    else:
                        drain_col(0, 1024)

            if loop_n:
                ws = emit_weight_loads()
                with tc.For_i(0, loop_n, 1):
                    _emit_body(ws)
            else:
                x_sb = alloc_x()
                ws = emit_weight_loads()
                _emit_body(ws, x_pre=x_sb)

    _split_waits(nc)
    return nc


_NC = None


def _get_nc():
    global _NC
    if _NC is None:
        _install_compat()
        _NC = build_nc()
    return _NC


def _pack_rows(W, o):
    # [C, o] -> [128, 4*o]: row c = a*128 + p  ->  partition p, chunk a
    return np.ascontiguousarray(
        W.reshape(4, 128, o).transpose(1, 0, 2).reshape(128, 4 * o)
    )


def _host_prep(x, gamma, beta, w_qkv, b_qkv, w_proj, b_proj):
    x = np.asarray(x, np.float32)
    gamma = np.asarray(gamma, np.float32)
    beta = np.asarray(beta, np.float32)
    w_qkv = np.asarray(w_qkv, np.float32)
    b_qkv = np.asarray(b_qkv, np.float32)
    w_proj = np.asarray(w_proj, np.float32)
    b_proj = np.asarray(b_proj, np.float32)

    s2 = 1.0 / np.sqrt(CH)  # attention scale applied to q AND k => s^2 on q
    Wg = w_qkv * gamma[None, :]
    bb = w_qkv @ beta + b_qkv
    Wg = Wg.copy()
    Wg[0:C] *= s2
    bb = bb.copy()
    bb[0:C] *= s2

    WDT = F8 if DEFAULT_CFG["fp8"] else BF16
    shared = {
        "wqk": _pack_rows(np.ascontiguousarray(Wg[0 : 2 * C].T), 2 * C).astype(WDT),
        "wv": _pack_rows(np.ascontiguousarray(Wg[2 * C : 3 * C].T), C).astype(WDT),
        "wp": _pack_rows(np.ascontiguousarray(w_proj.T), C).astype(WDT),
        "bqk": np.ascontiguousarray(bb[0 : 2 * C].reshape(8, 128).T).astype(
            np.float32
        ),
        "bvb": np.broadcast_to(bb[2 * C : 3 * C].reshape(1, C), (128, C)).astype(
            BF16
        ),
        "bp": np.ascontiguousarray(b_proj.reshape(4, 128).T).astype(np.float32),
        "ind": (np.arange(128)[:, None] // 16 == np.arange(8)[None, :]).astype(
            np.float32
        ),
        "indT": (np.arange(128)[None, :] // 16 == np.arange(8)[:, None]).astype(
            np.float32
        ),
    }
    in_maps = []
    for b in range(N_CORES):
        m = dict(shared)
        m["x"] = _pack_rows(x[b].reshape(C, L), L).astype(BF16)
        in_maps.append(m)
    return in_maps


def run_spmd(in_maps, trace=False):
    from concourse.bass_utils import run_bass_kernel_spmd

    nc = _get_nc()
    return run_bass_kernel_spmd(
        nc, in_maps, core_ids=list(range(N_CORES)), trace=trace
    )


def kernel(x, gamma, beta, w_qkv, b_qkv, w_proj, b_proj):
    _install_compat()
    in_maps = _host_prep(x, gamma, beta, w_qkv, b_qkv, w_proj, b_proj)
    res = run_spmd(in_maps, trace=False)
    out = np.stack(
        [res.results[c]["out"].reshape(C, 32, 32) for c in range(N_CORES)]
    ).astype(np.float32)
    return out
